# revision 11
# baseline (speedup 1.0000x reference)
"""Trainium2 Bass kernel for nn_ConvNat (2x NeighborhoodAttention2D + dwconv + linear).

Strategy (8 NeuronCores, SPMD):
  - Dense masked attention instead of the 31x31 gather: every query attends to all
    1296 keys; a host-precomputed log-domain bias (rpb value in-window, -20 outside)
    is added to the logits via an identity-matmul accumulate into PSUM, then exp on
    the scalar engine.  No max-subtraction needed (logits are tiny); the denominator
    comes from an extra ones-column appended to V.
  - Channel-major ("transposed") layouts throughout so no device transposes are
    needed on the critical path; projections are affine-folded (bias rows appended
    to the stationary operands).
  - Cores shard the 1296 queries into 8 bands of 162.  K/V are computed replicated;
    the NAT1 output band is AllGathered between the layers.
  - The dwconv and its bias, proj2 bias, and lin bias are all folded into the final
    linear (host-precomputed 9 tap matrices + one combined matrix).
"""

import os
import numpy as np

HEADS = 4
K = 31
C = 64
DH = 16
H = W = 36
N = H * W            # 1296 tokens
NCORES = 8
Q = N // NCORES      # 162 queries per core
NCH = 11             # n-chunks of 128 (1296 -> 1408 padded)
NPAD = NCH * 128
MASKVAL = -20.0

_CACHE = {}


def _build_bias(rpb, t0):
    """Log-domain dense bias for queries [t0, t0+Q): shape (HEADS, NCH, 128, Q)."""
    n = np.arange(NPAD)
    r = np.minimum(n // 36, 35)          # clamp pad rows (masked anyway)
    c = n % 36 if True else None
    c = n % 36
    valid_n = (n < N)
    t = np.arange(t0, t0 + Q)
    i = t // 36
    j = t % 36
    ri = np.clip(i - K // 2, 0, H - K)   # (Q,)
    cj = np.clip(j - K // 2, 0, W - K)
    # in-window mask: ri <= r <= ri+30, cj <= c <= cj+30
    rm = (r[:, None] >= ri[None, :]) & (r[:, None] <= ri[None, :] + K - 1)
    cm = (c[:, None] >= cj[None, :]) & (c[:, None] <= cj[None, :] + K - 1)
    m = rm & cm & valid_n[:, None]       # (NPAD, Q)
    rrel = np.clip(r[:, None] - i[None, :] + (K - 1), 0, 2 * K - 2)
    crel = np.clip(c[:, None] - j[None, :] + (K - 1), 0, 2 * K - 2)
    bias = rpb[:, rrel, crel]            # (HEADS, NPAD, Q)
    bias = np.where(m[None], bias, MASKVAL).astype(np.float32)
    return np.ascontiguousarray(bias.reshape(HEADS, NCH, 128, Q))


def _interleaved_wk(qkv_w, qkv_b, off, scale=1.0):
    """[65, 128] stationary: cols 32h+0..16 = head-h rows (off+16h+d) of qkv_w^T."""
    wt = np.zeros((C + 1, 128), np.float32)
    for h in range(HEADS):
        rows = qkv_w[off + 16 * h: off + 16 * h + 16]          # (16, 64)
        wt[:C, 32 * h: 32 * h + 16] = rows.T * scale
        wt[C, 32 * h: 32 * h + 16] = qkv_b[off + 16 * h: off + 16 * h + 16] * scale
    return wt


def _prep(inputs):
    x = np.asarray(inputs['x'], np.float32).reshape(N, C)
    p = {}
    xT = np.zeros((C + 1, NPAD), np.float32)
    xT[:C, :N] = x.T
    xT[C, :N] = 1.0
    p['xT'] = xT
    scale = DH ** -0.5
    for l, sfx in ((1, '1'), (2, '2')):
        qkv_w = np.asarray(inputs[f'qkv_w{sfx}'], np.float32)
        qkv_b = np.asarray(inputs[f'qkv_b{sfx}'], np.float32)
        p[f'wq{l}'] = _interleaved_wk(qkv_w, qkv_b, 0, scale)
        p[f'wk{l}'] = _interleaved_wk(qkv_w, qkv_b, C)
        wv = np.zeros((C + 1, C), np.float32)
        wv[:C] = qkv_w[2 * C:].T
        wv[C] = qkv_b[2 * C:]
        p[f'wv{l}'] = wv
    proj_w1 = np.asarray(inputs['proj_w1'], np.float32)
    proj_b1 = np.asarray(inputs['proj_b1'], np.float32)
    pr1 = np.zeros((C + 1, C), np.float32)
    pr1[:C] = proj_w1.T
    pr1[C] = proj_b1
    p['proj1'] = pr1
    lin_w = np.asarray(inputs['lin_w'], np.float32)
    lin_b = np.asarray(inputs['lin_b'], np.float32)
    proj_w2 = np.asarray(inputs['proj_w2'], np.float32)
    proj_b2 = np.asarray(inputs['proj_b2'], np.float32)
    dw_w = np.asarray(inputs['dw_w'], np.float32)               # (64, 1, 3, 3)
    dw_b = np.asarray(inputs['dw_b'], np.float32)
    m2 = np.zeros((C + 1, C), np.float32)
    m2[:C] = (lin_w @ proj_w2).T
    m2[C] = lin_w @ proj_b2 + lin_w @ dw_b + lin_b
    p['m2p'] = m2
    mtap = np.zeros((9, C, C), np.float32)
    for di in range(3):
        for dj in range(3):
            mtap[di * 3 + dj] = (lin_w * dw_w[None, :, 0, di, dj]).T  # diag(w)@lin_w^T
    p['mtap'] = np.ascontiguousarray(mtap)
    p['identity'] = np.eye(128, dtype=np.float32)
    # per-core tensors
    x_img = x.reshape(H, W, C).transpose(2, 0, 1)               # (64, 36, 36)
    xpad = np.zeros((C, H + 2, W + 2), np.float32)
    xpad[:, 1:-1, 1:-1] = x_img
    rpb1 = np.asarray(inputs['rpb1'], np.float32)
    rpb2 = np.asarray(inputs['rpb2'], np.float32)
    percore = []
    for core in range(NCORES):
        t0 = core * Q
        d = {}
        xq = np.zeros((C + 1, Q), np.float32)
        xq[:C] = x[t0:t0 + Q].T
        xq[C] = 1.0
        d['xq'] = xq
        xdw = np.zeros((9, C, Q), np.float32)
        for di in range(3):
            for dj in range(3):
                sh = xpad[:, di:di + H, dj:dj + W].reshape(C, N)
                xdw[di * 3 + dj] = sh[:, t0:t0 + Q]
        d['xdw'] = xdw
        d['b1'] = _build_bias(rpb1, t0)
        d['b2'] = _build_bias(rpb2, t0)
        percore.append(d)
    return p, percore


def _build_program():
    import concourse.bass as bass
    import concourse.bacc as bacc
    import concourse.tile as tile
    from concourse import mybir
    f32 = mybir.dt.float32
    AF = mybir.ActivationFunctionType

    nc = bacc.Bacc("TRN2", target_bir_lowering=False, debug=False,
                   num_devices=NCORES)

    # ---- dram I/O ----
    di = {}
    for name, shape in [
        ('xT', [C + 1, NPAD]), ('xq', [C + 1, Q]), ('xdw', [9, C, Q]),
        ('wq1', [C + 1, 128]), ('wk1', [C + 1, 128]), ('wv1', [C + 1, C]),
        ('wq2', [C + 1, 128]), ('wk2', [C + 1, 128]), ('wv2', [C + 1, C]),
        ('proj1', [C + 1, C]), ('m2p', [C + 1, C]), ('mtap', [9, C, C]),
        ('identity', [128, 128]),
        ('b1', [HEADS, NCH, 128, Q]), ('b2', [HEADS, NCH, 128, Q]),
    ]:
        di[name] = nc.dram_tensor(name, shape, f32, kind="ExternalInput")
    out_d = nc.dram_tensor('out', [Q, C], f32, kind="ExternalOutput")
    cc_in = nc.dram_tensor('cc_in', [C, Q], f32)
    cc_out = nc.dram_tensor('cc_out', [NCORES, C, Q], f32, addr_space="Shared")

    with tile.TileContext(nc) as tc:
        with (
            tc.tile_pool(name="const", bufs=1) as cpool,
            tc.tile_pool(name="work", bufs=2) as wpool,
            tc.tile_pool(name="ps_big", bufs=2, space="PSUM") as psb,
            tc.tile_pool(name="ps_small", bufs=2, space="PSUM") as pss,
        ):
            # ---- load constants ----
            def load(name, shape, src_ap):
                t = cpool.tile(shape, f32, name=name)
                nc.sync.dma_start(t[:], src_ap)
                return t

            id_sb = load('id_sb', [128, 128], di['identity'][:])
            w_sb = {}
            for name in ['wq1', 'wk1', 'wq2', 'wk2']:
                w_sb[name] = load(name + '_sb', [C + 1, 128], di[name][:])
            for name in ['wv1', 'wv2', 'proj1', 'm2p']:
                w_sb[name] = load(name + '_sb', [C + 1, C], di[name][:])
            mtap_sb = cpool.tile([C, 9 * C], f32, name='mtap_sb')
            nc.sync.dma_start(mtap_sb[:], di['mtap'].ap().rearrange("t a b -> a t b"))
            xdw_sb = cpool.tile([C, 9 * Q], f32, name='xdw_sb')
            nc.sync.dma_start(xdw_sb[:], di['xdw'].ap().rearrange("t c q -> c t q"))
            b_sb = {}
            for l in (1, 2):
                b_sb[l] = cpool.tile([128, HEADS * NCH * Q], f32, name=f'b{l}_sb')
                nc.sync.dma_start(
                    b_sb[l][:], di[f'b{l}'].ap().rearrange("h nb p q -> p h nb q"))

            # xT' (ones row and zero pad baked in on host)
            xT = cpool.tile([C + 1, NPAD], f32, name='xT')
            nc.sync.dma_start(xT[:], di['xT'][:])
            xqT = cpool.tile([C + 1, Q], f32, name='xqT')
            nc.sync.dma_start(xqT[:], di['xq'][:])

            # preload exp table with a tiny op
            dummy = cpool.tile([1, 1], f32, name='dummy')
            nc.vector.memset(dummy[:], 0.0)
            dummy2 = cpool.tile([1, 1], f32, name='dummy2')
            nc.scalar.activation(dummy2[:], dummy[:], AF.Exp)

            x2T = cpool.tile([C + 1, NPAD], f32, name='x2T')
            nc.vector.memset(x2T[:, N:], 0.0)
            nc.vector.memset(x2T[C:C + 1, :N], 1.0)

            y1T = cpool.tile([C + 1, Q], f32, name='y1T')
            nc.vector.memset(y1T[C:C + 1, :], 1.0)

            def nat_layer(l, srcT, src_qT):
                """srcT: [65, NPAD] AP (full tokens, ch-major, ones row, zero pad);
                src_qT: [65, Q] AP.  Returns attnT' [65, Q] tile (ones row set)."""
                wq, wk, wv = w_sb[f'wq{l}'], w_sb[f'wk{l}'], w_sb[f'wv{l}']
                # q projection -> [128, Q], head h at partitions 32h..32h+16
                ps_q = pss.tile([128, 512], f32, name='ps_q', tag='mm')
                nc.tensor.matmul(ps_q[:, :Q], wq[:], src_qT, start=True, stop=True)
                qT = wpool.tile([128, Q], f32, name='qT')
                nc.scalar.copy(qT[:], ps_q[:, :Q])
                # k projection -> kT [128, NPAD]
                kT = wpool.tile([128, NPAD], f32, name='kT')
                for jb, (s0, sz) in enumerate([(0, 512), (512, 512), (1024, 272)]):
                    ps_k = pss.tile([128, 512], f32, name='ps_k', tag='mm')
                    nc.tensor.matmul(ps_k[:, :sz], wk[:], srcT[:, s0:s0 + sz],
                                     start=True, stop=True)
                    nc.scalar.copy(kT[:, s0:s0 + sz], ps_k[:, :sz])
                # v projection -> VV [128, nb*68 + 17h + d], ones col at 17h+16
                VV = wpool.tile([128, NCH * 68], f32, name='VV')
                VVr = VV[:].rearrange("p (nb g d) -> p nb g d", g=HEADS, d=17)
                nc.vector.memset(VV[:], 0.0)
                nc.vector.memset(VVr[:, :, :, 16:17], 1.0)
                for nb in range(NCH):
                    nv = 128 if nb < NCH - 1 else N - 128 * (NCH - 1)
                    ps_v = pss.tile([128, 512], f32, name='ps_v', tag='mm')
                    nc.tensor.matmul(ps_v[:nv, :C],
                                     srcT[:, 128 * nb:128 * nb + nv],
                                     wv[:], start=True, stop=True)
                    nc.vector.tensor_copy(
                        VVr[:nv, nb, :, 0:16],
                        ps_v[:nv, :C].rearrange("p (g d) -> p g d", d=16))
                # S^T = K Q^T per head, + bias, exp -> PT
                PT = wpool.tile([128, HEADS * NCH * Q], f32, name='PT')
                PTr = PT[:].rearrange("p (h nb q) -> p h nb q", h=HEADS, nb=NCH)
                Br = b_sb[l][:].rearrange("p (h nb q) -> p h nb q", h=HEADS, nb=NCH)
                for h in range(HEADS):
                    for g0, gn in ((0, 6), (6, 5)):
                        ps_s = psb.tile([128, 1024], f32, name='ps_s', tag='s')
                        for i in range(gn):
                            nb = g0 + i
                            nv = 128 if nb < NCH - 1 else N - 128 * (NCH - 1)
                            col = (i // 3) * 512 + (i % 3) * Q
                            nc.tensor.matmul(
                                ps_s[:nv, col:col + Q],
                                kT[32 * h:32 * h + 16, 128 * nb:128 * nb + nv],
                                qT[32 * h:32 * h + 16, :],
                                start=(i % 3 == 0), stop=False,
                                skip_group_check=True, tile_position=(32 * h, 0))
                        for b in range((gn + 2) // 3):
                            cnt = min(3, gn - 3 * b) * Q
                            nc.tensor.matmul(
                                ps_s[:, 512 * b:512 * b + cnt],
                                id_sb[:],
                                Br[:, h, g0 + 3 * b:g0 + 3 * b + min(3, gn - 3 * b), :],
                                start=False, stop=(b == (gn - 1) // 3),
                                skip_group_check=True)
                        # exp from PSUM -> PT (per bank to keep APs simple)
                        for b in range((gn + 2) // 3):
                            nbs = min(3, gn - 3 * b)
                            nc.scalar.activation(
                                PTr[:, h, g0 + 3 * b:g0 + 3 * b + nbs, :],
                                ps_s[:, 512 * b:512 * b + nbs * Q],
                                AF.Exp)
                # PV: O^T[17h+..] accumulated over chunks
                ps_o = pss.tile([128, 512], f32, name='ps_o', tag='pair')
                for h in range(HEADS):
                    for nb in range(NCH):
                        nc.tensor.matmul(
                            ps_o[32 * h:32 * h + 17, :Q],
                            VVr[:, nb, h, :],
                            PTr[:, h, nb, :],
                            start=(nb == 0), stop=(nb == NCH - 1),
                            skip_group_check=True, tile_position=(0, 32 * h))
                # normalize: transpose -> divide -> transpose back
                o_sb = wpool.tile([128, Q], f32, name='o_sb')
                nc.vector.tensor_copy(o_sb[:], ps_o[:, :Q])
                ps_t0 = pss.tile([128, 512], f32, name='ps_t0', tag='pair')
                ps_t1 = pss.tile([128, 512], f32, name='ps_t1', tag='pair')
                nc.tensor.transpose(ps_t0[:, :128], o_sb[:, 0:128], id_sb[:])
                nc.tensor.transpose(ps_t1[:34, :128], o_sb[:, 128:Q], id_sb[:])
                rec = wpool.tile([128, 8], f32, name='rec')
                t0v = ps_t0[:, :128].rearrange("p (h d) -> p h d", d=32)
                t1v = ps_t1[:34, :128].rearrange("p (h d) -> p h d", d=32)
                nc.vector.reciprocal(rec[:, 0:4], t0v[:, :, 16:17])
                nc.vector.reciprocal(rec[:34, 4:8], t1v[:, :, 16:17])
                aq0 = wpool.tile([128, C], f32, name='aq0')
                aq1 = wpool.tile([34, C], f32, name='aq1')
                for h in range(HEADS):
                    nc.vector.tensor_scalar_mul(
                        aq0[:, 16 * h:16 * h + 16],
                        ps_t0[:, 32 * h:32 * h + 16], rec[:, h:h + 1])
                    nc.vector.tensor_scalar_mul(
                        aq1[:, 16 * h:16 * h + 16],
                        ps_t1[:34, 32 * h:32 * h + 16], rec[:34, 4 + h:5 + h])
                ps_a0 = pss.tile([128, 512], f32, name='ps_a0', tag='pair')
                ps_a1 = pss.tile([128, 512], f32, name='ps_a1', tag='pair')
                nc.tensor.transpose(ps_a0[:C, :128], aq0[:], id_sb[:])
                nc.tensor.transpose(ps_a1[:C, :34], aq1[:], id_sb[:34, :34])
                attnT = wpool.tile([C + 1, Q], f32, name=f'attnT{l}')
                nc.vector.memset(attnT[C:C + 1, :], 1.0)
                nc.scalar.copy(attnT[:C, 0:128], ps_a0[:C, :128])
                nc.scalar.copy(attnT[:C, 128:Q], ps_a1[:C, :34])
                return attnT

            # ---------------- layer 1 ----------------
            attnT1 = nat_layer(1, xT[:], xqT[:])
            ps_y = pss.tile([128, 512], f32, name='ps_y', tag='pair')
            nc.tensor.matmul(ps_y[:C, :Q], w_sb['proj1'][:], attnT1[:],
                             start=True, stop=True)
            nc.scalar.copy(y1T[:C, :], ps_y[:C, :Q])
            # all-gather y1 band
            nc.sync.dma_start(cc_in[:], y1T[:C, :])
            nc.gpsimd.collective_compute(
                "AllGather", mybir.AluOpType.bypass,
                replica_groups=[list(range(NCORES))],
                ins=[cc_in.ap().opt()], outs=[cc_out.ap().opt()])
            nc.sync.dma_start(x2T[:C, :N],
                              cc_out.ap().rearrange("r c q -> c r q"))
            # ---------------- layer 2 ----------------
            attnT2 = nat_layer(2, x2T[:], y1T[:])
            # final: z^T = m2p @ attnT2' + sum_tap mtap @ xdw
            ps_z = pss.tile([128, 512], f32, name='ps_z', tag='pair')
            nc.tensor.matmul(ps_z[:C, :Q], w_sb['m2p'][:], attnT2[:],
                             start=True, stop=False, skip_group_check=True)
            for t in range(9):
                nc.tensor.matmul(ps_z[:C, :Q],
                                 mtap_sb[:, C * t:C * t + C],
                                 xdw_sb[:, Q * t:Q * t + Q],
                                 start=False, stop=(t == 8), skip_group_check=True)
            z_sb = wpool.tile([C, Q], f32, name='z_sb')
            nc.scalar.copy(z_sb[:], ps_z[:C, :Q])
            ps_f0 = pss.tile([128, 512], f32, name='ps_f0', tag='pair')
            ps_f1 = pss.tile([128, 512], f32, name='ps_f1', tag='pair')
            nc.tensor.transpose(ps_f0[:, :C], z_sb[:, 0:128], id_sb[:C, :C])
            nc.tensor.transpose(ps_f1[:34, :C], z_sb[:, 128:Q], id_sb[:C, :C])
            zo0 = wpool.tile([128, C], f32, name='zo0')
            zo1 = wpool.tile([34, C], f32, name='zo1')
            nc.vector.tensor_copy(zo0[:], ps_f0[:, :C])
            nc.vector.tensor_copy(zo1[:], ps_f1[:34, :C])
            nc.sync.dma_start(out_d[0:128, :], zo0[:])
            nc.sync.dma_start(out_d[128:Q, :], zo1[:])

    nc.finalize()
    return nc


def kernel(**inputs) -> np.ndarray:
    from concourse.bass_utils import run_bass_kernel_spmd
    if 'nc' not in _CACHE:
        _CACHE['nc'] = _build_program()
    nc = _CACHE['nc']
    shared, percore = _prep(inputs)
    in_maps = []
    for core in range(NCORES):
        m = dict(shared)
        m.update(percore[core])
        in_maps.append(m)
    res = run_bass_kernel_spmd(nc, in_maps, core_ids=list(range(NCORES)))
    outs = [res.results[c]['out'] for c in range(NCORES)]
    full = np.concatenate(outs, axis=0).reshape(1, N, C)
    return full.astype(np.float32)


if __name__ == '__main__':
    import reference
    inputs = reference.setup_inputs()
    inputs = {k: np.asarray(v) for k, v in inputs.items()}
    got = kernel(**inputs)
    print("kernel output", got.shape, got.dtype)


# revision 14
# speedup vs baseline: 1.8511x; 1.8511x over previous
"""Trainium2 Bass kernel for nn_ConvNat (2x NeighborhoodAttention2D + dwconv + linear).

Strategy (8 NeuronCores, SPMD):
  - Dense masked attention instead of the 31x31 gather: every query attends to all
    1296 keys; a host-precomputed log-domain bias (rpb value in-window, -20 outside)
    is added to the logits via an identity-matmul accumulate into PSUM, then exp on
    the scalar engine.  No max-subtraction needed (logits are tiny); the denominator
    comes from an extra ones-column appended to V.
  - Channel-major ("transposed") layouts throughout so no device transposes are
    needed on the critical path; projections are affine-folded (bias rows appended
    to the stationary operands).
  - Cores shard the 1296 queries into 8 bands of 162.  K/V are computed replicated;
    the NAT1 output band is AllGathered between the layers.
  - The dwconv and its bias, proj2 bias, and lin bias are all folded into the final
    linear (host-precomputed 9 tap matrices + one combined matrix).
"""

import os
import numpy as np
import ml_dtypes

BF16 = ml_dtypes.bfloat16

HEADS = 4
K = 31
C = 64
DH = 16
H = W = 36
N = H * W            # 1296 tokens
NCORES = 8
Q = N // NCORES      # 162 queries per core
NCH = 11             # n-chunks of 128 (1296 -> 1408 padded)
NPAD = NCH * 128
MASKVAL = -20.0

_CACHE = {}


def _build_bias(rpb, t0):
    """Log-domain dense bias for queries [t0, t0+Q): shape (HEADS, NCH, 128, Q)."""
    n = np.arange(NPAD)
    r = np.minimum(n // 36, 35)          # clamp pad rows (masked anyway)
    c = n % 36 if True else None
    c = n % 36
    valid_n = (n < N)
    t = np.arange(t0, t0 + Q)
    i = t // 36
    j = t % 36
    ri = np.clip(i - K // 2, 0, H - K)   # (Q,)
    cj = np.clip(j - K // 2, 0, W - K)
    # in-window mask: ri <= r <= ri+30, cj <= c <= cj+30
    rm = (r[:, None] >= ri[None, :]) & (r[:, None] <= ri[None, :] + K - 1)
    cm = (c[:, None] >= cj[None, :]) & (c[:, None] <= cj[None, :] + K - 1)
    m = rm & cm & valid_n[:, None]       # (NPAD, Q)
    rrel = np.clip(r[:, None] - i[None, :] + (K - 1), 0, 2 * K - 2)
    crel = np.clip(c[:, None] - j[None, :] + (K - 1), 0, 2 * K - 2)
    bias = rpb[:, rrel, crel]            # (HEADS, NPAD, Q)
    bias = np.where(m[None], bias, MASKVAL).astype(np.float32)
    return np.ascontiguousarray(bias.reshape(HEADS, NCH, 128, Q))


def _interleaved_wk(qkv_w, qkv_b, off, scale=1.0):
    """[65, 128] stationary: cols 32h+0..16 = head-h rows (off+16h+d) of qkv_w^T."""
    wt = np.zeros((C + 1, 128), np.float32)
    for h in range(HEADS):
        rows = qkv_w[off + 16 * h: off + 16 * h + 16]          # (16, 64)
        wt[:C, 32 * h: 32 * h + 16] = rows.T * scale
        wt[C, 32 * h: 32 * h + 16] = qkv_b[off + 16 * h: off + 16 * h + 16] * scale
    return wt


def _bf(a):
    return np.ascontiguousarray(np.asarray(a, np.float32).astype(BF16))


def _prep(inputs):
    x = np.asarray(inputs['x'], np.float32).reshape(N, C)
    p = {}
    xT = np.zeros((C + 1, NPAD), np.float32)
    xT[:C, :N] = x.T
    xT[C, :N] = 1.0
    p['xT'] = xT
    scale = DH ** -0.5
    for l, sfx in ((1, '1'), (2, '2')):
        qkv_w = np.asarray(inputs[f'qkv_w{sfx}'], np.float32)
        qkv_b = np.asarray(inputs[f'qkv_b{sfx}'], np.float32)
        p[f'wq{l}'] = _interleaved_wk(qkv_w, qkv_b, 0, scale)
        p[f'wk{l}'] = _interleaved_wk(qkv_w, qkv_b, C)
        wv = np.zeros((C + 1, C), np.float32)
        wv[:C] = qkv_w[2 * C:].T
        wv[C] = qkv_b[2 * C:]
        p[f'wv{l}'] = wv
    proj_w1 = np.asarray(inputs['proj_w1'], np.float32)
    proj_b1 = np.asarray(inputs['proj_b1'], np.float32)
    pr1 = np.zeros((C + 1, C), np.float32)
    pr1[:C] = proj_w1.T
    pr1[C] = proj_b1
    p['proj1'] = pr1
    lin_w = np.asarray(inputs['lin_w'], np.float32)
    lin_b = np.asarray(inputs['lin_b'], np.float32)
    proj_w2 = np.asarray(inputs['proj_w2'], np.float32)
    proj_b2 = np.asarray(inputs['proj_b2'], np.float32)
    dw_w = np.asarray(inputs['dw_w'], np.float32)               # (64, 1, 3, 3)
    dw_b = np.asarray(inputs['dw_b'], np.float32)
    m2 = np.zeros((C + 1, C), np.float32)
    m2[:C] = (lin_w @ proj_w2).T
    m2[C] = lin_w @ proj_b2 + lin_w @ dw_b + lin_b
    p['m2p'] = m2
    mtap = np.zeros((9, C, C), np.float32)
    for di in range(3):
        for dj in range(3):
            mtap[di * 3 + dj] = (lin_w * dw_w[None, :, 0, di, dj]).T  # diag(w)@lin_w^T
    p['mtap'] = np.ascontiguousarray(mtap)
    p['identity'] = np.eye(128, dtype=np.float32)
    # per-core tensors
    x_img = x.reshape(H, W, C).transpose(2, 0, 1)               # (64, 36, 36)
    xpad = np.zeros((C, H + 2, W + 2), np.float32)
    xpad[:, 1:-1, 1:-1] = x_img
    rpb1 = np.asarray(inputs['rpb1'], np.float32)
    rpb2 = np.asarray(inputs['rpb2'], np.float32)
    percore = []
    for core in range(NCORES):
        t0 = core * Q
        d = {}
        xq = np.zeros((C + 1, Q), np.float32)
        xq[:C] = x[t0:t0 + Q].T
        xq[C] = 1.0
        d['xq'] = xq
        xdw = np.zeros((9, C, Q), np.float32)
        for di in range(3):
            for dj in range(3):
                sh = xpad[:, di:di + H, dj:dj + W].reshape(C, N)
                xdw[di * 3 + dj] = sh[:, t0:t0 + Q]
        d['xdw'] = xdw
        d['b1'] = _build_bias(rpb1, t0)
        d['b2'] = _build_bias(rpb2, t0)
        percore.append(d)
    p = {k: _bf(v) for k, v in p.items()}
    percore = [{k: _bf(v) for k, v in d.items()} for d in percore]
    return p, percore


def _build_program():
    import concourse.bass as bass
    import concourse.bacc as bacc
    import concourse.tile as tile
    from concourse import mybir
    f32 = mybir.dt.float32
    bf16 = mybir.dt.bfloat16
    AF = mybir.ActivationFunctionType

    nc = bacc.Bacc("TRN2", target_bir_lowering=False, debug=False,
                   num_devices=NCORES)

    # ---- dram I/O ----
    di = {}
    for name, shape in [
        ('xT', [C + 1, NPAD]), ('xq', [C + 1, Q]), ('xdw', [9, C, Q]),
        ('wq1', [C + 1, 128]), ('wk1', [C + 1, 128]), ('wv1', [C + 1, C]),
        ('wq2', [C + 1, 128]), ('wk2', [C + 1, 128]), ('wv2', [C + 1, C]),
        ('proj1', [C + 1, C]), ('m2p', [C + 1, C]), ('mtap', [9, C, C]),
        ('identity', [128, 128]),
        ('b1', [HEADS, NCH, 128, Q]), ('b2', [HEADS, NCH, 128, Q]),
    ]:
        di[name] = nc.dram_tensor(name, shape, bf16, kind="ExternalInput")
    out_d = nc.dram_tensor('out', [Q, C], f32, kind="ExternalOutput")
    cc_in = nc.dram_tensor('cc_in', [C, Q], bf16)
    cc_out = nc.dram_tensor('cc_out', [NCORES, C, Q], bf16, addr_space="Shared")

    with tile.TileContext(nc) as tc:
        with (
            tc.tile_pool(name="const", bufs=1) as cpool,
            tc.tile_pool(name="work", bufs=2) as wpool,
            tc.tile_pool(name="ps_big", bufs=4, space="PSUM") as psb,
            tc.tile_pool(name="ps_small", bufs=2, space="PSUM") as pss,
        ):
            # ---- load constants ----
            def load(name, shape, src_ap):
                t = cpool.tile(shape, bf16, name=name)
                nc.sync.dma_start(t[:], src_ap)
                return t

            id_sb = load('id_sb', [128, 128], di['identity'][:])
            w_sb = {}
            for name in ['wq1', 'wk1', 'wq2', 'wk2']:
                w_sb[name] = load(name + '_sb', [C + 1, 128], di[name][:])
            for name in ['wv1', 'wv2', 'proj1', 'm2p']:
                w_sb[name] = load(name + '_sb', [C + 1, C], di[name][:])
            mtap_sb = cpool.tile([C, 9 * C], bf16, name='mtap_sb')
            nc.sync.dma_start(mtap_sb[:], di['mtap'].ap().rearrange("t a b -> a t b"))
            xdw_sb = cpool.tile([C, 9 * Q], bf16, name='xdw_sb')
            nc.sync.dma_start(xdw_sb[:], di['xdw'].ap().rearrange("t c q -> c t q"))
            b_sb = {}
            for l in (1, 2):
                b_sb[l] = cpool.tile([128, HEADS * NCH * Q], bf16, name=f'b{l}_sb')
                nc.sync.dma_start(
                    b_sb[l][:], di[f'b{l}'].ap().rearrange("h nb p q -> p h nb q"))

            # xT' (ones row and zero pad baked in on host)
            xT = cpool.tile([C + 1, NPAD], bf16, name='xT')
            nc.sync.dma_start(xT[:], di['xT'][:])
            xqT = cpool.tile([C + 1, Q], bf16, name='xqT')
            nc.sync.dma_start(xqT[:], di['xq'][:])

            # preload exp table with a tiny op
            dummy = cpool.tile([1, 1], f32, name='dummy')
            nc.vector.memset(dummy[:], 0.0)
            dummy2 = cpool.tile([1, 1], f32, name='dummy2')
            nc.scalar.activation(dummy2[:], dummy[:], AF.Exp)

            x2T = cpool.tile([C + 1, NPAD], bf16, name='x2T')
            nc.vector.memset(x2T[:, N:], 0.0)
            nc.vector.memset(x2T[C:C + 1, :N], 1.0)

            y1T = cpool.tile([C + 1, Q], bf16, name='y1T')
            nc.vector.memset(y1T[C:C + 1, :], 1.0)

            def nat_layer(l, srcT, src_qT):
                """srcT: [65, NPAD] AP (full tokens, ch-major, ones row, zero pad);
                src_qT: [65, Q] AP.  Returns attnT' [65, Q] tile (ones row set)."""
                wq, wk, wv = w_sb[f'wq{l}'], w_sb[f'wk{l}'], w_sb[f'wv{l}']
                # q projection -> [128, Q], head h at partitions 32h..32h+16
                ps_q = pss.tile([128, 512], f32, name='ps_q', tag='mm')
                nc.tensor.matmul(ps_q[:, :Q], wq[:], src_qT, start=True, stop=True)
                qT = wpool.tile([128, Q], bf16, name='qT')
                nc.scalar.copy(qT[:], ps_q[:, :Q])
                # k projection -> kT [128, NPAD]
                kT = wpool.tile([128, NPAD], bf16, name='kT')
                for jb, (s0, sz) in enumerate([(0, 512), (512, 512), (1024, 272)]):
                    ps_k = pss.tile([128, 512], f32, name='ps_k', tag='mm')
                    nc.tensor.matmul(ps_k[:, :sz], wk[:], srcT[:, s0:s0 + sz],
                                     start=True, stop=True)
                    nc.scalar.copy(kT[:, s0:s0 + sz], ps_k[:, :sz])
                # v projection -> VV [128, nb*68 + 17h + d], ones col at 17h+16
                VV = wpool.tile([128, NCH * 68], bf16, name='VV')
                VVr = VV[:].rearrange("p (nb g d) -> p nb g d", g=HEADS, d=17)
                nc.vector.memset(VV[:], 0.0)
                nc.vector.memset(VVr[:, :, :, 16:17], 1.0)
                for nb in range(NCH):
                    nv = 128 if nb < NCH - 1 else N - 128 * (NCH - 1)
                    ps_v = pss.tile([128, 512], f32, name='ps_v', tag='mm')
                    nc.tensor.matmul(ps_v[:nv, :C],
                                     srcT[:, 128 * nb:128 * nb + nv],
                                     wv[:], start=True, stop=True)
                    nc.vector.tensor_copy(
                        VVr[:nv, nb, :, 0:16],
                        ps_v[:nv, :C].rearrange("p (g d) -> p g d", d=16))
                # S^T = K Q^T per head, + bias, exp -> PT
                PT = wpool.tile([128, HEADS * NCH * Q], bf16, name='PT')
                PTr = PT[:].rearrange("p (h nb q) -> p h nb q", h=HEADS, nb=NCH)
                Br = b_sb[l][:].rearrange("p (h nb q) -> p h nb q", h=HEADS, nb=NCH)
                for g0, gn in ((0, 3), (3, 3), (6, 3), (9, 2)):
                    stiles = []
                    for h in range(HEADS):
                        ps_s = psb.tile([128, 512], f32, name='ps_s', tag='s')
                        stiles.append(ps_s)
                    for i in range(gn):
                        nb = g0 + i
                        nv = 128 if nb < NCH - 1 else N - 128 * (NCH - 1)
                        for h in range(HEADS):
                            nc.tensor.matmul(
                                stiles[h][:nv, i * Q:i * Q + Q],
                                kT[32 * h:32 * h + 16, 128 * nb:128 * nb + nv],
                                qT[32 * h:32 * h + 16, :],
                                start=(i == 0), stop=False,
                                skip_group_check=True, tile_position=(32 * h, 0))
                    for h in range(HEADS):
                        nc.tensor.matmul(
                            stiles[h][:, 0:gn * Q],
                            id_sb[:],
                            Br[:, h, g0:g0 + gn, :],
                            start=False, stop=True,
                            skip_group_check=True)
                    for h in range(HEADS):
                        nc.scalar.activation(
                            PTr[:, h, g0:g0 + gn, :],
                            stiles[h][:, 0:gn * Q],
                            AF.Exp)
                # PV: O^T[17h+..] accumulated over chunks
                ps_o = pss.tile([128, 512], f32, name='ps_o', tag='pair')
                for h in range(HEADS):
                    for nb in range(NCH):
                        nc.tensor.matmul(
                            ps_o[32 * h:32 * h + 17, :Q],
                            VVr[:, nb, h, :],
                            PTr[:, h, nb, :],
                            start=(nb == 0), stop=(nb == NCH - 1),
                            skip_group_check=True, tile_position=(0, 32 * h))
                # normalize: transpose -> divide -> transpose back
                o_sb = wpool.tile([128, Q], bf16, name='o_sb')
                nc.vector.tensor_copy(o_sb[:], ps_o[:, :Q])
                ps_t0 = pss.tile([128, 512], bf16, name='ps_t0', tag='pair')
                ps_t1 = pss.tile([128, 512], bf16, name='ps_t1', tag='pair')
                nc.tensor.transpose(ps_t0[:, :128], o_sb[:, 0:128], id_sb[:])
                nc.tensor.transpose(ps_t1[:34, :128], o_sb[:, 128:Q], id_sb[:])
                rec = wpool.tile([128, 8], f32, name='rec')
                t0v = ps_t0[:, :128].rearrange("p (h d) -> p h d", d=32)
                t1v = ps_t1[:34, :128].rearrange("p (h d) -> p h d", d=32)
                nc.vector.reciprocal(rec[:, 0:4], t0v[:, :, 16:17])
                nc.vector.reciprocal(rec[:34, 4:8], t1v[:, :, 16:17])
                aq0 = wpool.tile([128, C], bf16, name='aq0')
                aq1 = wpool.tile([34, C], bf16, name='aq1')
                for h in range(HEADS):
                    nc.vector.tensor_scalar_mul(
                        aq0[:, 16 * h:16 * h + 16],
                        ps_t0[:, 32 * h:32 * h + 16], rec[:, h:h + 1])
                    nc.vector.tensor_scalar_mul(
                        aq1[:, 16 * h:16 * h + 16],
                        ps_t1[:34, 32 * h:32 * h + 16], rec[:34, 4 + h:5 + h])
                ps_a0 = pss.tile([128, 512], bf16, name='ps_a0', tag='pair')
                ps_a1 = pss.tile([128, 512], bf16, name='ps_a1', tag='pair')
                nc.tensor.transpose(ps_a0[:C, :128], aq0[:], id_sb[:])
                nc.tensor.transpose(ps_a1[:C, :34], aq1[:], id_sb[:34, :34])
                attnT = wpool.tile([C + 1, Q], bf16, name=f'attnT{l}')
                nc.vector.memset(attnT[C:C + 1, :], 1.0)
                nc.scalar.copy(attnT[:C, 0:128], ps_a0[:C, :128])
                nc.scalar.copy(attnT[:C, 128:Q], ps_a1[:C, :34])
                return attnT

            # ---------------- layer 1 ----------------
            attnT1 = nat_layer(1, xT[:], xqT[:])
            ps_y = pss.tile([128, 512], f32, name='ps_y', tag='pair')
            nc.tensor.matmul(ps_y[:C, :Q], w_sb['proj1'][:], attnT1[:],
                             start=True, stop=True)
            nc.scalar.copy(y1T[:C, :], ps_y[:C, :Q])
            # all-gather y1 band
            nc.sync.dma_start(cc_in[:], y1T[:C, :])
            nc.gpsimd.collective_compute(
                "AllGather", mybir.AluOpType.bypass,
                replica_groups=[list(range(NCORES))],
                ins=[cc_in.ap().opt()], outs=[cc_out.ap().opt()])
            nc.sync.dma_start(x2T[:C, :N],
                              cc_out.ap().rearrange("r c q -> c r q"))
            # ---------------- layer 2 ----------------
            attnT2 = nat_layer(2, x2T[:], y1T[:])
            # final: z^T = m2p @ attnT2' + sum_tap mtap @ xdw
            ps_z = pss.tile([128, 512], f32, name='ps_z', tag='pair')
            nc.tensor.matmul(ps_z[:C, :Q], w_sb['m2p'][:], attnT2[:],
                             start=True, stop=False, skip_group_check=True)
            for t in range(9):
                nc.tensor.matmul(ps_z[:C, :Q],
                                 mtap_sb[:, C * t:C * t + C],
                                 xdw_sb[:, Q * t:Q * t + Q],
                                 start=False, stop=(t == 8), skip_group_check=True)
            z_sb = wpool.tile([C, Q], bf16, name='z_sb')
            nc.scalar.copy(z_sb[:], ps_z[:C, :Q])
            ps_f0 = pss.tile([128, 512], bf16, name='ps_f0', tag='pair')
            ps_f1 = pss.tile([128, 512], bf16, name='ps_f1', tag='pair')
            nc.tensor.transpose(ps_f0[:, :C], z_sb[:, 0:128], id_sb[:C, :C])
            nc.tensor.transpose(ps_f1[:34, :C], z_sb[:, 128:Q], id_sb[:C, :C])
            zo0 = wpool.tile([128, C], f32, name='zo0')
            zo1 = wpool.tile([34, C], f32, name='zo1')
            nc.vector.tensor_copy(zo0[:], ps_f0[:, :C])
            nc.vector.tensor_copy(zo1[:], ps_f1[:34, :C])
            nc.sync.dma_start(out_d[0:128, :], zo0[:])
            nc.sync.dma_start(out_d[128:Q, :], zo1[:])

    nc.finalize()
    return nc


def kernel(**inputs) -> np.ndarray:
    from concourse.bass_utils import run_bass_kernel_spmd
    if 'nc' not in _CACHE:
        _CACHE['nc'] = _build_program()
    nc = _CACHE['nc']
    shared, percore = _prep(inputs)
    in_maps = []
    for core in range(NCORES):
        m = dict(shared)
        m.update(percore[core])
        in_maps.append(m)
    res = run_bass_kernel_spmd(nc, in_maps, core_ids=list(range(NCORES)))
    outs = [res.results[c]['out'] for c in range(NCORES)]
    full = np.concatenate(outs, axis=0).reshape(1, N, C)
    return full.astype(np.float32)


if __name__ == '__main__':
    import reference
    inputs = reference.setup_inputs()
    inputs = {k: np.asarray(v) for k, v in inputs.items()}
    got = kernel(**inputs)
    print("kernel output", got.shape, got.dtype)


# revision 15
# speedup vs baseline: 1.9732x; 1.0660x over previous
"""Trainium2 Bass kernel for nn_ConvNat (2x NeighborhoodAttention2D + dwconv + linear).

Strategy (8 NeuronCores, SPMD):
  - Dense masked attention instead of the 31x31 gather: every query attends to all
    1296 keys; a host-precomputed log-domain bias (rpb value in-window, -20 outside)
    is added to the logits via an identity-matmul accumulate into PSUM, then exp on
    the scalar engine.  No max-subtraction needed (logits are tiny); the denominator
    comes from an extra ones-column appended to V.
  - Channel-major ("transposed") layouts throughout so no device transposes are
    needed on the critical path; projections are affine-folded (bias rows appended
    to the stationary operands).
  - Cores shard the 1296 queries into 8 bands of 162.  K/V are computed replicated;
    the NAT1 output band is AllGathered between the layers.
  - The dwconv and its bias, proj2 bias, and lin bias are all folded into the final
    linear (host-precomputed 9 tap matrices + one combined matrix).
"""

import os
import numpy as np
import ml_dtypes

BF16 = ml_dtypes.bfloat16

HEADS = 4
K = 31
C = 64
DH = 16
H = W = 36
N = H * W            # 1296 tokens
NCORES = 8
Q = N // NCORES      # 162 queries per core
NCH = 11             # n-chunks of 128 (1296 -> 1408 padded)
NPAD = NCH * 128
MASKVAL = -20.0

_CACHE = {}


def _build_bias(rpb, t0):
    """Log-domain dense bias for queries [t0, t0+Q): shape (HEADS, NCH, 128, Q)."""
    n = np.arange(NPAD)
    r = np.minimum(n // 36, 35)          # clamp pad rows (masked anyway)
    c = n % 36 if True else None
    c = n % 36
    valid_n = (n < N)
    t = np.arange(t0, t0 + Q)
    i = t // 36
    j = t % 36
    ri = np.clip(i - K // 2, 0, H - K)   # (Q,)
    cj = np.clip(j - K // 2, 0, W - K)
    # in-window mask: ri <= r <= ri+30, cj <= c <= cj+30
    rm = (r[:, None] >= ri[None, :]) & (r[:, None] <= ri[None, :] + K - 1)
    cm = (c[:, None] >= cj[None, :]) & (c[:, None] <= cj[None, :] + K - 1)
    m = rm & cm & valid_n[:, None]       # (NPAD, Q)
    rrel = np.clip(r[:, None] - i[None, :] + (K - 1), 0, 2 * K - 2)
    crel = np.clip(c[:, None] - j[None, :] + (K - 1), 0, 2 * K - 2)
    bias = rpb[:, rrel, crel]            # (HEADS, NPAD, Q)
    bias = np.where(m[None], bias, MASKVAL).astype(np.float32)
    bias = bias.reshape(HEADS, NCH, 128, Q).transpose(2, 0, 1, 3)
    return np.ascontiguousarray(bias.reshape(128, HEADS * NCH * Q))


def _interleaved_wk(qkv_w, qkv_b, off, scale=1.0):
    """[65, 128] stationary: cols 32h+0..16 = head-h rows (off+16h+d) of qkv_w^T."""
    wt = np.zeros((C + 1, 128), np.float32)
    for h in range(HEADS):
        rows = qkv_w[off + 16 * h: off + 16 * h + 16]          # (16, 64)
        wt[:C, 32 * h: 32 * h + 16] = rows.T * scale
        wt[C, 32 * h: 32 * h + 16] = qkv_b[off + 16 * h: off + 16 * h + 16] * scale
    return wt


def _bf(a):
    return np.ascontiguousarray(np.asarray(a, np.float32).astype(BF16))


def _prep(inputs):
    x = np.asarray(inputs['x'], np.float32).reshape(N, C)
    p = {}
    xT = np.zeros((C + 1, NPAD), np.float32)
    xT[:C, :N] = x.T
    xT[C, :N] = 1.0
    p['xT'] = xT
    scale = DH ** -0.5
    for l, sfx in ((1, '1'), (2, '2')):
        qkv_w = np.asarray(inputs[f'qkv_w{sfx}'], np.float32)
        qkv_b = np.asarray(inputs[f'qkv_b{sfx}'], np.float32)
        p[f'wq{l}'] = _interleaved_wk(qkv_w, qkv_b, 0, scale)
        p[f'wk{l}'] = _interleaved_wk(qkv_w, qkv_b, C)
        wv = np.zeros((C + 1, C), np.float32)
        wv[:C] = qkv_w[2 * C:].T
        wv[C] = qkv_b[2 * C:]
        p[f'wv{l}'] = wv
    proj_w1 = np.asarray(inputs['proj_w1'], np.float32)
    proj_b1 = np.asarray(inputs['proj_b1'], np.float32)
    pr1 = np.zeros((C + 1, C), np.float32)
    pr1[:C] = proj_w1.T
    pr1[C] = proj_b1
    p['proj1'] = pr1
    lin_w = np.asarray(inputs['lin_w'], np.float32)
    lin_b = np.asarray(inputs['lin_b'], np.float32)
    proj_w2 = np.asarray(inputs['proj_w2'], np.float32)
    proj_b2 = np.asarray(inputs['proj_b2'], np.float32)
    dw_w = np.asarray(inputs['dw_w'], np.float32)               # (64, 1, 3, 3)
    dw_b = np.asarray(inputs['dw_b'], np.float32)
    m2 = np.zeros((C + 1, C), np.float32)
    m2[:C] = (lin_w @ proj_w2).T
    m2[C] = lin_w @ proj_b2 + lin_w @ dw_b + lin_b
    p['m2p'] = m2
    mtap = np.zeros((9, C, C), np.float32)
    for di in range(3):
        for dj in range(3):
            mtap[di * 3 + dj] = (lin_w * dw_w[None, :, 0, di, dj]).T  # diag(w)@lin_w^T
    p['mtap'] = np.ascontiguousarray(mtap)
    p['identity'] = np.eye(128, dtype=np.float32)
    # per-core tensors
    x_img = x.reshape(H, W, C).transpose(2, 0, 1)               # (64, 36, 36)
    xpad = np.zeros((C, H + 2, W + 2), np.float32)
    xpad[:, 1:-1, 1:-1] = x_img
    rpb1 = np.asarray(inputs['rpb1'], np.float32)
    rpb2 = np.asarray(inputs['rpb2'], np.float32)
    percore = []
    for core in range(NCORES):
        t0 = core * Q
        d = {}
        xq = np.zeros((C + 1, Q), np.float32)
        xq[:C] = x[t0:t0 + Q].T
        xq[C] = 1.0
        d['xq'] = xq
        xdw = np.zeros((9, C, Q), np.float32)
        for di in range(3):
            for dj in range(3):
                sh = xpad[:, di:di + H, dj:dj + W].reshape(C, N)
                xdw[di * 3 + dj] = sh[:, t0:t0 + Q]
        d['xdw'] = xdw
        d['b1'] = _build_bias(rpb1, t0)
        d['b2'] = _build_bias(rpb2, t0)
        percore.append(d)
    p = {k: _bf(v) for k, v in p.items()}
    percore = [{k: _bf(v) for k, v in d.items()} for d in percore]
    return p, percore


def _build_program():
    import concourse.bass as bass
    import concourse.bacc as bacc
    import concourse.tile as tile
    from concourse import mybir
    f32 = mybir.dt.float32
    bf16 = mybir.dt.bfloat16
    AF = mybir.ActivationFunctionType

    nc = bacc.Bacc("TRN2", target_bir_lowering=False, debug=False,
                   num_devices=NCORES)

    # ---- dram I/O ----
    di = {}
    for name, shape in [
        ('xT', [C + 1, NPAD]), ('xq', [C + 1, Q]), ('xdw', [9, C, Q]),
        ('wq1', [C + 1, 128]), ('wk1', [C + 1, 128]), ('wv1', [C + 1, C]),
        ('wq2', [C + 1, 128]), ('wk2', [C + 1, 128]), ('wv2', [C + 1, C]),
        ('proj1', [C + 1, C]), ('m2p', [C + 1, C]), ('mtap', [9, C, C]),
        ('identity', [128, 128]),
        ('b1', [128, HEADS * NCH * Q]), ('b2', [128, HEADS * NCH * Q]),
    ]:
        di[name] = nc.dram_tensor(name, shape, bf16, kind="ExternalInput")
    out_d = nc.dram_tensor('out', [Q, C], f32, kind="ExternalOutput")
    cc_in = nc.dram_tensor('cc_in', [C, Q], bf16)
    cc_out = nc.dram_tensor('cc_out', [NCORES, C, Q], bf16, addr_space="Shared")

    with tile.TileContext(nc) as tc:
        with (
            tc.tile_pool(name="const", bufs=1) as cpool,
            tc.tile_pool(name="work", bufs=2) as wpool,
            tc.tile_pool(name="ps_big", bufs=4, space="PSUM") as psb,
            tc.tile_pool(name="ps_small", bufs=2, space="PSUM") as pss,
        ):
            # ---- load constants ----
            def load(name, shape, src_ap):
                t = cpool.tile(shape, bf16, name=name)
                nc.sync.dma_start(t[:], src_ap)
                return t

            id_sb = load('id_sb', [128, 128], di['identity'][:])
            w_sb = {}
            for name in ['wq1', 'wk1', 'wq2', 'wk2']:
                w_sb[name] = load(name + '_sb', [C + 1, 128], di[name][:])
            for name in ['wv1', 'wv2', 'proj1', 'm2p']:
                w_sb[name] = load(name + '_sb', [C + 1, C], di[name][:])
            mtap_sb = cpool.tile([C, 9 * C], bf16, name='mtap_sb')
            nc.sync.dma_start(mtap_sb[:], di['mtap'].ap().rearrange("t a b -> a t b"))
            xdw_sb = cpool.tile([C, 9 * Q], bf16, name='xdw_sb')
            nc.sync.dma_start(xdw_sb[:], di['xdw'].ap().rearrange("t c q -> c t q"))
            b_sb = {}
            for l in (1, 2):
                b_sb[l] = cpool.tile([128, HEADS * NCH * Q], bf16, name=f'b{l}_sb')
                nc.sync.dma_start(b_sb[l][:], di[f'b{l}'][:])

            # xT' (ones row and zero pad baked in on host)
            xT = cpool.tile([C + 1, NPAD], bf16, name='xT')
            nc.sync.dma_start(xT[:], di['xT'][:])
            xqT = cpool.tile([C + 1, Q], bf16, name='xqT')
            nc.sync.dma_start(xqT[:], di['xq'][:])

            # preload exp table with a tiny op
            dummy = cpool.tile([1, 1], f32, name='dummy')
            nc.vector.memset(dummy[:], 0.0)
            dummy2 = cpool.tile([1, 1], f32, name='dummy2')
            nc.scalar.activation(dummy2[:], dummy[:], AF.Exp)

            x2T = cpool.tile([C + 1, NPAD], bf16, name='x2T')
            nc.vector.memset(x2T[:, N:], 0.0)
            nc.vector.memset(x2T[C:C + 1, :N], 1.0)

            y1T = cpool.tile([C + 1, Q], bf16, name='y1T')
            nc.vector.memset(y1T[C:C + 1, :], 1.0)

            def nat_layer(l, srcT, src_qT):
                """srcT: [65, NPAD] AP (full tokens, ch-major, ones row, zero pad);
                src_qT: [65, Q] AP.  Returns attnT' [65, Q] tile (ones row set)."""
                wq, wk, wv = w_sb[f'wq{l}'], w_sb[f'wk{l}'], w_sb[f'wv{l}']
                # q projection -> [128, Q], head h at partitions 32h..32h+16
                ps_q = pss.tile([128, 512], f32, name='ps_q', tag='mm')
                nc.tensor.matmul(ps_q[:, :Q], wq[:], src_qT, start=True, stop=True)
                qT = wpool.tile([128, Q], bf16, name='qT')
                nc.scalar.copy(qT[:], ps_q[:, :Q])
                # k projection -> kT [128, NPAD]
                kT = wpool.tile([128, NPAD], bf16, name='kT')
                for jb, (s0, sz) in enumerate([(0, 512), (512, 512), (1024, 272)]):
                    ps_k = pss.tile([128, 512], f32, name='ps_k', tag='mm')
                    nc.tensor.matmul(ps_k[:, :sz], wk[:], srcT[:, s0:s0 + sz],
                                     start=True, stop=True)
                    nc.scalar.copy(kT[:, s0:s0 + sz], ps_k[:, :sz])
                # v projection -> VV [128, nb*68 + 17h + d], ones col at 17h+16
                VV = wpool.tile([128, NCH * 68], bf16, name='VV')
                VVr = VV[:].rearrange("p (nb g d) -> p nb g d", g=HEADS, d=17)
                nc.vector.memset(VV[:], 0.0)
                nc.vector.memset(VVr[:, :, :, 16:17], 1.0)
                for nb in range(NCH):
                    nv = 128 if nb < NCH - 1 else N - 128 * (NCH - 1)
                    ps_v = pss.tile([128, 512], f32, name='ps_v', tag='mm')
                    nc.tensor.matmul(ps_v[:nv, :C],
                                     srcT[:, 128 * nb:128 * nb + nv],
                                     wv[:], start=True, stop=True)
                    nc.vector.tensor_copy(
                        VVr[:nv, nb, :, 0:16],
                        ps_v[:nv, :C].rearrange("p (g d) -> p g d", d=16))
                # S^T = K Q^T per head, + bias, exp -> PT
                PT = wpool.tile([128, HEADS * NCH * Q], bf16, name='PT')
                PTr = PT[:].rearrange("p (h nb q) -> p h nb q", h=HEADS, nb=NCH)
                Br = b_sb[l][:].rearrange("p (h nb q) -> p h nb q", h=HEADS, nb=NCH)
                for g0, gn in ((0, 3), (3, 3), (6, 3), (9, 2)):
                    stiles = []
                    for h in range(HEADS):
                        ps_s = psb.tile([128, 512], f32, name='ps_s', tag='s')
                        stiles.append(ps_s)
                    for i in range(gn):
                        nb = g0 + i
                        nv = 128 if nb < NCH - 1 else N - 128 * (NCH - 1)
                        for h in range(HEADS):
                            nc.tensor.matmul(
                                stiles[h][:nv, i * Q:i * Q + Q],
                                kT[32 * h:32 * h + 16, 128 * nb:128 * nb + nv],
                                qT[32 * h:32 * h + 16, :],
                                start=(i == 0), stop=False,
                                skip_group_check=True, tile_position=(32 * h, 0))
                    for h in range(HEADS):
                        nc.tensor.matmul(
                            stiles[h][:, 0:gn * Q],
                            id_sb[:],
                            Br[:, h, g0:g0 + gn, :],
                            start=False, stop=True,
                            skip_group_check=True)
                    for h in range(HEADS):
                        nc.scalar.activation(
                            PTr[:, h, g0:g0 + gn, :],
                            stiles[h][:, 0:gn * Q],
                            AF.Exp)
                # PV: O^T[17h+..] accumulated over chunks
                ps_o = pss.tile([128, 512], f32, name='ps_o', tag='pair')
                for h in range(HEADS):
                    for nb in range(NCH):
                        nc.tensor.matmul(
                            ps_o[32 * h:32 * h + 17, :Q],
                            VVr[:, nb, h, :],
                            PTr[:, h, nb, :],
                            start=(nb == 0), stop=(nb == NCH - 1),
                            skip_group_check=True, tile_position=(0, 32 * h))
                # normalize: transpose -> divide -> transpose back
                o_sb = wpool.tile([128, Q], bf16, name='o_sb')
                nc.vector.tensor_copy(o_sb[:], ps_o[:, :Q])
                ps_t0 = pss.tile([128, 512], bf16, name='ps_t0', tag='pair')
                ps_t1 = pss.tile([128, 512], bf16, name='ps_t1', tag='pair')
                nc.tensor.transpose(ps_t0[:, :128], o_sb[:, 0:128], id_sb[:])
                nc.tensor.transpose(ps_t1[:34, :128], o_sb[:, 128:Q], id_sb[:])
                rec = wpool.tile([128, 8], f32, name='rec')
                t0v = ps_t0[:, :128].rearrange("p (h d) -> p h d", d=32)
                t1v = ps_t1[:34, :128].rearrange("p (h d) -> p h d", d=32)
                nc.vector.reciprocal(rec[:, 0:4], t0v[:, :, 16:17])
                nc.vector.reciprocal(rec[:34, 4:8], t1v[:, :, 16:17])
                aq0 = wpool.tile([128, C], bf16, name='aq0')
                aq1 = wpool.tile([34, C], bf16, name='aq1')
                for h in range(HEADS):
                    nc.vector.tensor_scalar_mul(
                        aq0[:, 16 * h:16 * h + 16],
                        ps_t0[:, 32 * h:32 * h + 16], rec[:, h:h + 1])
                    nc.vector.tensor_scalar_mul(
                        aq1[:, 16 * h:16 * h + 16],
                        ps_t1[:34, 32 * h:32 * h + 16], rec[:34, 4 + h:5 + h])
                ps_a0 = pss.tile([128, 512], bf16, name='ps_a0', tag='pair')
                ps_a1 = pss.tile([128, 512], bf16, name='ps_a1', tag='pair')
                nc.tensor.transpose(ps_a0[:C, :128], aq0[:], id_sb[:])
                nc.tensor.transpose(ps_a1[:C, :34], aq1[:], id_sb[:34, :34])
                attnT = wpool.tile([C + 1, Q], bf16, name=f'attnT{l}')
                nc.vector.memset(attnT[C:C + 1, :], 1.0)
                nc.scalar.copy(attnT[:C, 0:128], ps_a0[:C, :128])
                nc.scalar.copy(attnT[:C, 128:Q], ps_a1[:C, :34])
                return attnT

            # ---------------- layer 1 ----------------
            attnT1 = nat_layer(1, xT[:], xqT[:])
            ps_y = pss.tile([128, 512], f32, name='ps_y', tag='pair')
            nc.tensor.matmul(ps_y[:C, :Q], w_sb['proj1'][:], attnT1[:],
                             start=True, stop=True)
            nc.scalar.copy(y1T[:C, :], ps_y[:C, :Q])
            # all-gather y1 band
            nc.sync.dma_start(cc_in[:], y1T[:C, :])
            nc.gpsimd.collective_compute(
                "AllGather", mybir.AluOpType.bypass,
                replica_groups=[list(range(NCORES))],
                ins=[cc_in.ap().opt()], outs=[cc_out.ap().opt()])
            nc.sync.dma_start(x2T[:C, :N],
                              cc_out.ap().rearrange("r c q -> c r q"))
            # ---------------- layer 2 ----------------
            attnT2 = nat_layer(2, x2T[:], y1T[:])
            # final: z^T = m2p @ attnT2' + sum_tap mtap @ xdw
            ps_z = pss.tile([128, 512], f32, name='ps_z', tag='pair')
            for t in range(9):
                nc.tensor.matmul(ps_z[:C, :Q],
                                 mtap_sb[:, C * t:C * t + C],
                                 xdw_sb[:, Q * t:Q * t + Q],
                                 start=(t == 0), stop=False, skip_group_check=True)
            nc.tensor.matmul(ps_z[:C, :Q], w_sb['m2p'][:], attnT2[:],
                             start=False, stop=True, skip_group_check=True)
            z_sb = wpool.tile([C, Q], bf16, name='z_sb')
            nc.scalar.copy(z_sb[:], ps_z[:C, :Q])
            ps_f0 = pss.tile([128, 512], bf16, name='ps_f0', tag='pair')
            ps_f1 = pss.tile([128, 512], bf16, name='ps_f1', tag='pair')
            nc.tensor.transpose(ps_f0[:, :C], z_sb[:, 0:128], id_sb[:C, :C])
            nc.tensor.transpose(ps_f1[:34, :C], z_sb[:, 128:Q], id_sb[:C, :C])
            zo0 = wpool.tile([128, C], f32, name='zo0')
            zo1 = wpool.tile([34, C], f32, name='zo1')
            nc.vector.tensor_copy(zo0[:], ps_f0[:, :C])
            nc.vector.tensor_copy(zo1[:], ps_f1[:34, :C])
            nc.sync.dma_start(out_d[0:128, :], zo0[:])
            nc.sync.dma_start(out_d[128:Q, :], zo1[:])

    nc.finalize()
    return nc


def kernel(**inputs) -> np.ndarray:
    from concourse.bass_utils import run_bass_kernel_spmd
    if 'nc' not in _CACHE:
        _CACHE['nc'] = _build_program()
    nc = _CACHE['nc']
    shared, percore = _prep(inputs)
    in_maps = []
    for core in range(NCORES):
        m = dict(shared)
        m.update(percore[core])
        in_maps.append(m)
    res = run_bass_kernel_spmd(nc, in_maps, core_ids=list(range(NCORES)))
    outs = [res.results[c]['out'] for c in range(NCORES)]
    full = np.concatenate(outs, axis=0).reshape(1, N, C)
    return full.astype(np.float32)


if __name__ == '__main__':
    import reference
    inputs = reference.setup_inputs()
    inputs = {k: np.asarray(v) for k, v in inputs.items()}
    got = kernel(**inputs)
    print("kernel output", got.shape, got.dtype)


# revision 16
# speedup vs baseline: 2.1232x; 1.0760x over previous
"""Trainium2 Bass kernel for nn_ConvNat (2x NeighborhoodAttention2D + dwconv + linear).

Strategy (8 NeuronCores, SPMD):
  - Dense masked attention instead of the 31x31 gather: every query attends to all
    1296 keys; a host-precomputed log-domain bias (rpb value in-window, -20 outside)
    is added to the logits via an identity-matmul accumulate into PSUM, then exp on
    the scalar engine.  No max-subtraction needed (logits are tiny); the denominator
    comes from an extra ones-column appended to V.
  - Channel-major ("transposed") layouts throughout so no device transposes are
    needed on the critical path; projections are affine-folded (bias rows appended
    to the stationary operands).
  - Cores shard the 1296 queries into 8 bands of 162.  K/V are computed replicated;
    the NAT1 output band is AllGathered between the layers.
  - The dwconv and its bias, proj2 bias, and lin bias are all folded into the final
    linear (host-precomputed 9 tap matrices + one combined matrix).
"""

import os
import numpy as np
import ml_dtypes

BF16 = ml_dtypes.bfloat16

HEADS = 4
K = 31
C = 64
DH = 16
H = W = 36
N = H * W            # 1296 tokens
NCORES = 8
Q = N // NCORES      # 162 queries per core
NCH = 11             # n-chunks of 128 (1296 -> 1408 padded)
NPAD = NCH * 128
MASKVAL = -20.0

_CACHE = {}


def _build_bias(rpb, t0):
    """Log-domain dense bias for queries [t0, t0+Q): shape (HEADS, NCH, 128, Q)."""
    n = np.arange(NPAD)
    r = np.minimum(n // 36, 35)          # clamp pad rows (masked anyway)
    c = n % 36 if True else None
    c = n % 36
    valid_n = (n < N)
    t = np.arange(t0, t0 + Q)
    i = t // 36
    j = t % 36
    ri = np.clip(i - K // 2, 0, H - K)   # (Q,)
    cj = np.clip(j - K // 2, 0, W - K)
    # in-window mask: ri <= r <= ri+30, cj <= c <= cj+30
    rm = (r[:, None] >= ri[None, :]) & (r[:, None] <= ri[None, :] + K - 1)
    cm = (c[:, None] >= cj[None, :]) & (c[:, None] <= cj[None, :] + K - 1)
    m = rm & cm & valid_n[:, None]       # (NPAD, Q)
    rrel = np.clip(r[:, None] - i[None, :] + (K - 1), 0, 2 * K - 2)
    crel = np.clip(c[:, None] - j[None, :] + (K - 1), 0, 2 * K - 2)
    bias = rpb[:, rrel, crel]            # (HEADS, NPAD, Q)
    bias = np.where(m[None], bias, MASKVAL).astype(np.float32)
    bias = bias.reshape(HEADS, NCH, 128, Q).transpose(2, 0, 1, 3)
    return np.ascontiguousarray(bias.reshape(128, HEADS * NCH * Q))


def _interleaved_wk(qkv_w, qkv_b, off, scale=1.0):
    """[65, 128] stationary: cols 32h+0..16 = head-h rows (off+16h+d) of qkv_w^T."""
    wt = np.zeros((C + 1, 128), np.float32)
    for h in range(HEADS):
        rows = qkv_w[off + 16 * h: off + 16 * h + 16]          # (16, 64)
        wt[:C, 32 * h: 32 * h + 16] = rows.T * scale
        wt[C, 32 * h: 32 * h + 16] = qkv_b[off + 16 * h: off + 16 * h + 16] * scale
    return wt


def _bf(a):
    return np.ascontiguousarray(np.asarray(a, np.float32).astype(BF16))


def _prep(inputs):
    x = np.asarray(inputs['x'], np.float32).reshape(N, C)
    p = {}
    xT = np.zeros((C + 1, NPAD), np.float32)
    xT[:C, :N] = x.T
    xT[C, :N] = 1.0
    p['xT'] = xT
    scale = DH ** -0.5
    for l, sfx in ((1, '1'), (2, '2')):
        qkv_w = np.asarray(inputs[f'qkv_w{sfx}'], np.float32)
        qkv_b = np.asarray(inputs[f'qkv_b{sfx}'], np.float32)
        p[f'wq{l}'] = _interleaved_wk(qkv_w, qkv_b, 0, scale)
        p[f'wk{l}'] = _interleaved_wk(qkv_w, qkv_b, C)
        wv = np.zeros((C + 1, C), np.float32)
        wv[:C] = qkv_w[2 * C:].T
        wv[C] = qkv_b[2 * C:]
        p[f'wv{l}'] = wv
    proj_w1 = np.asarray(inputs['proj_w1'], np.float32)
    proj_b1 = np.asarray(inputs['proj_b1'], np.float32)
    pr1 = np.zeros((C + 1, C), np.float32)
    pr1[:C] = proj_w1.T
    pr1[C] = proj_b1
    p['proj1'] = pr1
    lin_w = np.asarray(inputs['lin_w'], np.float32)
    lin_b = np.asarray(inputs['lin_b'], np.float32)
    proj_w2 = np.asarray(inputs['proj_w2'], np.float32)
    proj_b2 = np.asarray(inputs['proj_b2'], np.float32)
    dw_w = np.asarray(inputs['dw_w'], np.float32)               # (64, 1, 3, 3)
    dw_b = np.asarray(inputs['dw_b'], np.float32)
    m2 = np.zeros((C + 1, C), np.float32)
    m2[:C] = (lin_w @ proj_w2).T
    m2[C] = lin_w @ proj_b2 + lin_w @ dw_b + lin_b
    p['m2p'] = m2
    mtap = np.zeros((9, C, C), np.float32)
    for di in range(3):
        for dj in range(3):
            mtap[di * 3 + dj] = (lin_w * dw_w[None, :, 0, di, dj]).T  # diag(w)@lin_w^T
    p['mtap'] = np.ascontiguousarray(mtap)
    p['identity'] = np.eye(128, dtype=np.float32)
    # per-core tensors
    x_img = x.reshape(H, W, C).transpose(2, 0, 1)               # (64, 36, 36)
    xpad = np.zeros((C, H + 2, W + 2), np.float32)
    xpad[:, 1:-1, 1:-1] = x_img
    rpb1 = np.asarray(inputs['rpb1'], np.float32)
    rpb2 = np.asarray(inputs['rpb2'], np.float32)
    percore = []
    for core in range(NCORES):
        t0 = core * Q
        d = {}
        xq = np.zeros((C + 1, Q), np.float32)
        xq[:C] = x[t0:t0 + Q].T
        xq[C] = 1.0
        d['xq'] = xq
        xdw = np.zeros((9, C, Q), np.float32)
        for di in range(3):
            for dj in range(3):
                sh = xpad[:, di:di + H, dj:dj + W].reshape(C, N)
                xdw[di * 3 + dj] = sh[:, t0:t0 + Q]
        d['xdw'] = xdw
        d['b1'] = _build_bias(rpb1, t0)
        d['b2'] = _build_bias(rpb2, t0)
        percore.append(d)
    p = {k: _bf(v) for k, v in p.items()}
    percore = [{k: _bf(v) for k, v in d.items()} for d in percore]
    return p, percore


def _build_program():
    import concourse.bass as bass
    import concourse.bacc as bacc
    import concourse.tile as tile
    from concourse import mybir
    f32 = mybir.dt.float32
    bf16 = mybir.dt.bfloat16
    AF = mybir.ActivationFunctionType

    nc = bacc.Bacc("TRN2", target_bir_lowering=False, debug=False,
                   num_devices=NCORES)

    # ---- dram I/O ----
    di = {}
    for name, shape in [
        ('xT', [C + 1, NPAD]), ('xq', [C + 1, Q]), ('xdw', [9, C, Q]),
        ('wq1', [C + 1, 128]), ('wk1', [C + 1, 128]), ('wv1', [C + 1, C]),
        ('wq2', [C + 1, 128]), ('wk2', [C + 1, 128]), ('wv2', [C + 1, C]),
        ('proj1', [C + 1, C]), ('m2p', [C + 1, C]), ('mtap', [9, C, C]),
        ('identity', [128, 128]),
        ('b1', [128, HEADS * NCH * Q]), ('b2', [128, HEADS * NCH * Q]),
    ]:
        di[name] = nc.dram_tensor(name, shape, bf16, kind="ExternalInput")
    out_d = nc.dram_tensor('out', [Q, C], f32, kind="ExternalOutput")
    cc_in = nc.dram_tensor('cc_in', [C, Q], bf16)
    cc_out = nc.dram_tensor('cc_out', [NCORES, C, Q], bf16, addr_space="Shared")

    with tile.TileContext(nc) as tc:
        with (
            tc.tile_pool(name="const", bufs=1) as cpool,
            tc.tile_pool(name="work", bufs=2) as wpool,
            tc.tile_pool(name="ps_big", bufs=4, space="PSUM") as psb,
            tc.tile_pool(name="ps_small", bufs=2, space="PSUM") as pss,
        ):
            # ---- load constants ----
            def load(name, shape, src_ap):
                t = cpool.tile(shape, bf16, name=name)
                nc.sync.dma_start(t[:], src_ap)
                return t

            id_sb = load('id_sb', [128, 128], di['identity'][:])
            w_sb = {}
            for name in ['wq1', 'wk1', 'wq2', 'wk2']:
                w_sb[name] = load(name + '_sb', [C + 1, 128], di[name][:])
            for name in ['wv1', 'wv2', 'proj1', 'm2p']:
                w_sb[name] = load(name + '_sb', [C + 1, C], di[name][:])
            mtap_sb = cpool.tile([C, 9 * C], bf16, name='mtap_sb')
            nc.sync.dma_start(mtap_sb[:], di['mtap'].ap().rearrange("t a b -> a t b"))
            xdw_sb = cpool.tile([C, 9 * Q], bf16, name='xdw_sb')
            nc.sync.dma_start(xdw_sb[:], di['xdw'].ap().rearrange("t c q -> c t q"))
            b_sb = {}
            for l in (1, 2):
                b_sb[l] = cpool.tile([128, HEADS * NCH * Q], bf16, name=f'b{l}_sb')
                nc.sync.dma_start(b_sb[l][:], di[f'b{l}'][:])

            # xT' (ones row and zero pad baked in on host)
            xT = cpool.tile([C + 1, NPAD], bf16, name='xT')
            nc.sync.dma_start(xT[:], di['xT'][:])
            xqT = cpool.tile([C + 1, Q], bf16, name='xqT')
            nc.sync.dma_start(xqT[:], di['xq'][:])

            # preload exp table with a tiny op
            dummy = cpool.tile([1, 1], f32, name='dummy')
            nc.vector.memset(dummy[:], 0.0)
            dummy2 = cpool.tile([1, 1], f32, name='dummy2')
            nc.scalar.activation(dummy2[:], dummy[:], AF.Exp)

            x2T = cpool.tile([C + 1, NPAD], bf16, name='x2T')
            nc.vector.memset(x2T[:, N:], 0.0)
            nc.vector.memset(x2T[C:C + 1, :N], 1.0)

            y1T = cpool.tile([C + 1, Q], bf16, name='y1T')
            nc.vector.memset(y1T[C:C + 1, :], 1.0)

            def nat_layer(l, srcT, src_qT):
                """srcT: [65, NPAD] AP (full tokens, ch-major, ones row, zero pad);
                src_qT: [65, Q] AP.  Returns attnT' [65, Q] tile (ones row set)."""
                wq, wk, wv = w_sb[f'wq{l}'], w_sb[f'wk{l}'], w_sb[f'wv{l}']
                # q projection -> [128, Q], head h at partitions 32h..32h+16
                ps_q = pss.tile([128, 512], f32, name='ps_q', tag='mm')
                nc.tensor.matmul(ps_q[:, :Q], wq[:], src_qT, start=True, stop=True)
                qT = wpool.tile([128, Q], bf16, name='qT')
                nc.vector.tensor_copy(qT[:], ps_q[:, :Q])
                # k projection -> kT [128, NPAD]
                kT = wpool.tile([128, NPAD], bf16, name='kT')
                for jb, (s0, sz) in enumerate([(0, 512), (512, 512), (1024, 272)]):
                    ps_k = pss.tile([128, 512], f32, name='ps_k', tag='mm')
                    nc.tensor.matmul(ps_k[:, :sz], wk[:], srcT[:, s0:s0 + sz],
                                     start=True, stop=True)
                    nc.vector.tensor_copy(kT[:, s0:s0 + sz], ps_k[:, :sz])
                # v projection -> VV [128, nb*68 + 17h + d], ones col at 17h+16
                VV = wpool.tile([128, NCH * 68], bf16, name='VV')
                VVr = VV[:].rearrange("p (nb g d) -> p nb g d", g=HEADS, d=17)
                nc.vector.memset(VV[:], 0.0)
                nc.vector.memset(VVr[:, :, :, 16:17], 1.0)
                for nb in range(NCH):
                    nv = 128 if nb < NCH - 1 else N - 128 * (NCH - 1)
                    ps_v = pss.tile([128, 512], f32, name='ps_v', tag='mm')
                    nc.tensor.matmul(ps_v[:nv, :C],
                                     srcT[:, 128 * nb:128 * nb + nv],
                                     wv[:], start=True, stop=True)
                    nc.vector.tensor_copy(
                        VVr[:nv, nb, :, 0:16],
                        ps_v[:nv, :C].rearrange("p (g d) -> p g d", d=16))
                # S^T = K Q^T per head, + bias, exp -> PT
                PT = wpool.tile([128, HEADS * NCH * Q], bf16, name='PT')
                PTr = PT[:].rearrange("p (h nb q) -> p h nb q", h=HEADS, nb=NCH)
                Br = b_sb[l][:].rearrange("p (h nb q) -> p h nb q", h=HEADS, nb=NCH)
                ps_o = pss.tile([128, 512], f32, name='ps_o', tag='pair')
                for g0, gn in ((0, 3), (3, 3), (6, 3), (9, 2)):
                    stiles = []
                    for h in range(HEADS):
                        ps_s = psb.tile([128, 512], f32, name='ps_s', tag='s')
                        stiles.append(ps_s)
                    for i in range(gn):
                        nb = g0 + i
                        nv = 128 if nb < NCH - 1 else N - 128 * (NCH - 1)
                        for h in range(HEADS):
                            nc.tensor.matmul(
                                stiles[h][:nv, i * Q:i * Q + Q],
                                kT[32 * h:32 * h + 16, 128 * nb:128 * nb + nv],
                                qT[32 * h:32 * h + 16, :],
                                start=(i == 0), stop=False,
                                skip_group_check=True, tile_position=(32 * h, 0))
                    for h in range(HEADS):
                        nc.tensor.matmul(
                            stiles[h][:, 0:gn * Q],
                            id_sb[:],
                            Br[:, h, g0:g0 + gn, :],
                            start=False, stop=True,
                            skip_group_check=True)
                    for h in range(HEADS):
                        nc.scalar.activation(
                            PTr[:, h, g0:g0 + gn, :],
                            stiles[h][:, 0:gn * Q],
                            AF.Exp)
                    # PV for this group's chunks (overlaps next group's S-mms)
                    for i in range(gn):
                        nb = g0 + i
                        for h in range(HEADS):
                            nc.tensor.matmul(
                                ps_o[32 * h:32 * h + 17, :Q],
                                VVr[:, nb, h, :],
                                PTr[:, h, nb, :],
                                start=(nb == 0), stop=(nb == NCH - 1),
                                skip_group_check=True, tile_position=(0, 32 * h))
                # normalize: transpose -> divide -> transpose back
                o_sb = wpool.tile([128, Q], bf16, name='o_sb')
                nc.vector.tensor_copy(o_sb[:], ps_o[:, :Q])
                ps_t0 = pss.tile([128, 512], bf16, name='ps_t0', tag='pair')
                ps_t1 = pss.tile([128, 512], bf16, name='ps_t1', tag='pair')
                nc.tensor.transpose(ps_t0[:, :128], o_sb[:, 0:128], id_sb[:])
                nc.tensor.transpose(ps_t1[:34, :128], o_sb[:, 128:Q], id_sb[:])
                rec = wpool.tile([128, 8], f32, name='rec')
                t0v = ps_t0[:, :128].rearrange("p (h d) -> p h d", d=32)
                t1v = ps_t1[:34, :128].rearrange("p (h d) -> p h d", d=32)
                nc.vector.reciprocal(rec[:, 0:4], t0v[:, :, 16:17])
                nc.vector.reciprocal(rec[:34, 4:8], t1v[:, :, 16:17])
                aq0 = wpool.tile([128, C], bf16, name='aq0')
                aq1 = wpool.tile([34, C], bf16, name='aq1')
                for h in range(HEADS):
                    nc.vector.tensor_scalar_mul(
                        aq0[:, 16 * h:16 * h + 16],
                        ps_t0[:, 32 * h:32 * h + 16], rec[:, h:h + 1])
                    nc.vector.tensor_scalar_mul(
                        aq1[:, 16 * h:16 * h + 16],
                        ps_t1[:34, 32 * h:32 * h + 16], rec[:34, 4 + h:5 + h])
                ps_a0 = pss.tile([128, 512], bf16, name='ps_a0', tag='pair')
                ps_a1 = pss.tile([128, 512], bf16, name='ps_a1', tag='pair')
                nc.tensor.transpose(ps_a0[:C, :128], aq0[:], id_sb[:])
                nc.tensor.transpose(ps_a1[:C, :34], aq1[:], id_sb[:34, :34])
                attnT = wpool.tile([C + 1, Q], bf16, name=f'attnT{l}')
                nc.vector.memset(attnT[C:C + 1, :], 1.0)
                nc.scalar.copy(attnT[:C, 0:128], ps_a0[:C, :128])
                nc.scalar.copy(attnT[:C, 128:Q], ps_a1[:C, :34])
                return attnT

            # ---------------- layer 1 ----------------
            attnT1 = nat_layer(1, xT[:], xqT[:])
            ps_y = pss.tile([128, 512], f32, name='ps_y', tag='pair')
            nc.tensor.matmul(ps_y[:C, :Q], w_sb['proj1'][:], attnT1[:],
                             start=True, stop=True)
            nc.scalar.copy(y1T[:C, :], ps_y[:C, :Q])
            # all-gather y1 band
            nc.sync.dma_start(cc_in[:], y1T[:C, :])
            nc.gpsimd.collective_compute(
                "AllGather", mybir.AluOpType.bypass,
                replica_groups=[list(range(NCORES))],
                ins=[cc_in.ap().opt()], outs=[cc_out.ap().opt()])
            nc.sync.dma_start(x2T[:C, :N],
                              cc_out.ap().rearrange("r c q -> c r q"))
            # ---------------- layer 2 ----------------
            attnT2 = nat_layer(2, x2T[:], y1T[:])
            # final: z^T = m2p @ attnT2' + sum_tap mtap @ xdw
            ps_z = pss.tile([128, 512], f32, name='ps_z', tag='pair')
            for t in range(9):
                nc.tensor.matmul(ps_z[:C, :Q],
                                 mtap_sb[:, C * t:C * t + C],
                                 xdw_sb[:, Q * t:Q * t + Q],
                                 start=(t == 0), stop=False, skip_group_check=True)
            nc.tensor.matmul(ps_z[:C, :Q], w_sb['m2p'][:], attnT2[:],
                             start=False, stop=True, skip_group_check=True)
            z_sb = wpool.tile([C, Q], bf16, name='z_sb')
            nc.scalar.copy(z_sb[:], ps_z[:C, :Q])
            ps_f0 = pss.tile([128, 512], bf16, name='ps_f0', tag='pair')
            ps_f1 = pss.tile([128, 512], bf16, name='ps_f1', tag='pair')
            nc.tensor.transpose(ps_f0[:, :C], z_sb[:, 0:128], id_sb[:C, :C])
            nc.tensor.transpose(ps_f1[:34, :C], z_sb[:, 128:Q], id_sb[:C, :C])
            zo0 = wpool.tile([128, C], f32, name='zo0')
            zo1 = wpool.tile([34, C], f32, name='zo1')
            nc.vector.tensor_copy(zo0[:], ps_f0[:, :C])
            nc.vector.tensor_copy(zo1[:], ps_f1[:34, :C])
            nc.sync.dma_start(out_d[0:128, :], zo0[:])
            nc.sync.dma_start(out_d[128:Q, :], zo1[:])

    nc.finalize()
    return nc


def kernel(**inputs) -> np.ndarray:
    from concourse.bass_utils import run_bass_kernel_spmd
    if 'nc' not in _CACHE:
        _CACHE['nc'] = _build_program()
    nc = _CACHE['nc']
    shared, percore = _prep(inputs)
    in_maps = []
    for core in range(NCORES):
        m = dict(shared)
        m.update(percore[core])
        in_maps.append(m)
    res = run_bass_kernel_spmd(nc, in_maps, core_ids=list(range(NCORES)))
    outs = [res.results[c]['out'] for c in range(NCORES)]
    full = np.concatenate(outs, axis=0).reshape(1, N, C)
    return full.astype(np.float32)


if __name__ == '__main__':
    import reference
    inputs = reference.setup_inputs()
    inputs = {k: np.asarray(v) for k, v in inputs.items()}
    got = kernel(**inputs)
    print("kernel output", got.shape, got.dtype)


# revision 17
# speedup vs baseline: 2.2378x; 1.0540x over previous
"""Trainium2 Bass kernel for nn_ConvNat (2x NeighborhoodAttention2D + dwconv + linear).

Strategy (8 NeuronCores, SPMD):
  - Dense masked attention instead of the 31x31 gather: every query attends to all
    1296 keys; a host-precomputed log-domain bias (rpb value in-window, -20 outside)
    is added to the logits via an identity-matmul accumulate into PSUM, then exp on
    the scalar engine.  No max-subtraction needed (logits are tiny); the denominator
    comes from an extra ones-column appended to V.
  - Channel-major ("transposed") layouts throughout so no device transposes are
    needed on the critical path; projections are affine-folded (bias rows appended
    to the stationary operands).
  - Cores shard the 1296 queries into 8 bands of 162.  K/V are computed replicated;
    the NAT1 output band is AllGathered between the layers.
  - The dwconv and its bias, proj2 bias, and lin bias are all folded into the final
    linear (host-precomputed 9 tap matrices + one combined matrix).
"""

import os
import numpy as np
import ml_dtypes

BF16 = ml_dtypes.bfloat16

HEADS = 4
K = 31
C = 64
DH = 16
H = W = 36
N = H * W            # 1296 tokens
NCORES = 8
Q = N // NCORES      # 162 queries per core
NCH = 11             # n-chunks of 128 (1296 -> 1408 padded)
NPAD = NCH * 128
MASKVAL = -20.0

_CACHE = {}


def _build_bias(rpb, t0):
    """Log-domain dense bias for queries [t0, t0+Q): shape (HEADS, NCH, 128, Q)."""
    n = np.arange(NPAD)
    r = np.minimum(n // 36, 35)          # clamp pad rows (masked anyway)
    c = n % 36 if True else None
    c = n % 36
    valid_n = (n < N)
    t = np.arange(t0, t0 + Q)
    i = t // 36
    j = t % 36
    ri = np.clip(i - K // 2, 0, H - K)   # (Q,)
    cj = np.clip(j - K // 2, 0, W - K)
    # in-window mask: ri <= r <= ri+30, cj <= c <= cj+30
    rm = (r[:, None] >= ri[None, :]) & (r[:, None] <= ri[None, :] + K - 1)
    cm = (c[:, None] >= cj[None, :]) & (c[:, None] <= cj[None, :] + K - 1)
    m = rm & cm & valid_n[:, None]       # (NPAD, Q)
    rrel = np.clip(r[:, None] - i[None, :] + (K - 1), 0, 2 * K - 2)
    crel = np.clip(c[:, None] - j[None, :] + (K - 1), 0, 2 * K - 2)
    bias = rpb[:, rrel, crel]            # (HEADS, NPAD, Q)
    bias = np.where(m[None], bias, MASKVAL).astype(np.float32)
    bias = bias.reshape(HEADS, NCH, 128, Q).transpose(2, 0, 1, 3)
    return np.ascontiguousarray(bias.reshape(128, HEADS * NCH * Q))


def _interleaved_wk(qkv_w, qkv_b, off, scale=1.0):
    """[65, 128] stationary: cols 32h+0..16 = head-h rows (off+16h+d) of qkv_w^T."""
    wt = np.zeros((C + 1, 128), np.float32)
    for h in range(HEADS):
        rows = qkv_w[off + 16 * h: off + 16 * h + 16]          # (16, 64)
        wt[:C, 32 * h: 32 * h + 16] = rows.T * scale
        wt[C, 32 * h: 32 * h + 16] = qkv_b[off + 16 * h: off + 16 * h + 16] * scale
    return wt


def _bf(a):
    return np.ascontiguousarray(np.asarray(a, np.float32).astype(BF16))


def _prep(inputs):
    x = np.asarray(inputs['x'], np.float32).reshape(N, C)
    p = {}
    xT = np.zeros((C + 1, NPAD), np.float32)
    xT[:C, :N] = x.T
    xT[C, :N] = 1.0
    p['xT'] = xT
    scale = DH ** -0.5
    for l, sfx in ((1, '1'), (2, '2')):
        qkv_w = np.asarray(inputs[f'qkv_w{sfx}'], np.float32)
        qkv_b = np.asarray(inputs[f'qkv_b{sfx}'], np.float32)
        p[f'wq{l}'] = _interleaved_wk(qkv_w, qkv_b, 0, scale)
        p[f'wk{l}'] = _interleaved_wk(qkv_w, qkv_b, C)
        wv = np.zeros((C + 1, C), np.float32)
        wv[:C] = qkv_w[2 * C:].T
        wv[C] = qkv_b[2 * C:]
        p[f'wv{l}'] = wv
    proj_w1 = np.asarray(inputs['proj_w1'], np.float32)
    proj_b1 = np.asarray(inputs['proj_b1'], np.float32)
    pr1 = np.zeros((C + 1, C), np.float32)
    pr1[:C] = proj_w1.T
    pr1[C] = proj_b1
    p['proj1'] = pr1
    lin_w = np.asarray(inputs['lin_w'], np.float32)
    lin_b = np.asarray(inputs['lin_b'], np.float32)
    proj_w2 = np.asarray(inputs['proj_w2'], np.float32)
    proj_b2 = np.asarray(inputs['proj_b2'], np.float32)
    dw_w = np.asarray(inputs['dw_w'], np.float32)               # (64, 1, 3, 3)
    dw_b = np.asarray(inputs['dw_b'], np.float32)
    m2 = np.zeros((C + 1, C), np.float32)
    m2[:C] = (lin_w @ proj_w2).T
    m2[C] = lin_w @ proj_b2 + lin_w @ dw_b + lin_b
    p['m2p'] = m2
    mtap = np.zeros((9, C, C), np.float32)
    for di in range(3):
        for dj in range(3):
            mtap[di * 3 + dj] = (lin_w * dw_w[None, :, 0, di, dj]).T  # diag(w)@lin_w^T
    p['mtap'] = np.ascontiguousarray(mtap)
    p['identity'] = np.eye(128, dtype=np.float32)
    # per-core tensors
    x_img = x.reshape(H, W, C).transpose(2, 0, 1)               # (64, 36, 36)
    xpad = np.zeros((C, H + 2, W + 2), np.float32)
    xpad[:, 1:-1, 1:-1] = x_img
    rpb1 = np.asarray(inputs['rpb1'], np.float32)
    rpb2 = np.asarray(inputs['rpb2'], np.float32)
    percore = []
    for core in range(NCORES):
        t0 = core * Q
        d = {}
        xq = np.zeros((C + 1, Q), np.float32)
        xq[:C] = x[t0:t0 + Q].T
        xq[C] = 1.0
        d['xq'] = xq
        xdw = np.zeros((9, C, Q), np.float32)
        for di in range(3):
            for dj in range(3):
                sh = xpad[:, di:di + H, dj:dj + W].reshape(C, N)
                xdw[di * 3 + dj] = sh[:, t0:t0 + Q]
        d['xdw'] = xdw
        d['b1'] = _build_bias(rpb1, t0)
        d['b2'] = _build_bias(rpb2, t0)
        percore.append(d)
    p = {k: _bf(v) for k, v in p.items()}
    percore = [{k: _bf(v) for k, v in d.items()} for d in percore]
    return p, percore


def _build_program():
    import concourse.bass as bass
    import concourse.bacc as bacc
    import concourse.tile as tile
    from concourse import mybir
    f32 = mybir.dt.float32
    bf16 = mybir.dt.bfloat16
    AF = mybir.ActivationFunctionType

    nc = bacc.Bacc("TRN2", target_bir_lowering=False, debug=False,
                   num_devices=NCORES)

    # ---- dram I/O ----
    di = {}
    for name, shape in [
        ('xT', [C + 1, NPAD]), ('xq', [C + 1, Q]), ('xdw', [9, C, Q]),
        ('wq1', [C + 1, 128]), ('wk1', [C + 1, 128]), ('wv1', [C + 1, C]),
        ('wq2', [C + 1, 128]), ('wk2', [C + 1, 128]), ('wv2', [C + 1, C]),
        ('proj1', [C + 1, C]), ('m2p', [C + 1, C]), ('mtap', [9, C, C]),
        ('identity', [128, 128]),
        ('b1', [128, HEADS * NCH * Q]), ('b2', [128, HEADS * NCH * Q]),
    ]:
        di[name] = nc.dram_tensor(name, shape, bf16, kind="ExternalInput")
    out_d = nc.dram_tensor('out', [Q, C], f32, kind="ExternalOutput")
    cc_in = nc.dram_tensor('cc_in', [C, Q], bf16)
    cc_out = nc.dram_tensor('cc_out', [NCORES, C, Q], bf16, addr_space="Shared")

    with tile.TileContext(nc) as tc:
        with (
            tc.tile_pool(name="const", bufs=1) as cpool,
            tc.tile_pool(name="work", bufs=2) as wpool,
            tc.tile_pool(name="ps_big", bufs=4, space="PSUM") as psb,
            tc.tile_pool(name="ps_small", bufs=2, space="PSUM") as pss,
        ):
            # ---- load constants ----
            def load(name, shape, src_ap):
                t = cpool.tile(shape, bf16, name=name)
                nc.sync.dma_start(t[:], src_ap)
                return t

            id_sb = load('id_sb', [128, 128], di['identity'][:])
            w_sb = {}
            for name in ['wq1', 'wk1', 'wq2', 'wk2']:
                w_sb[name] = load(name + '_sb', [C + 1, 128], di[name][:])
            for name in ['wv1', 'wv2', 'proj1', 'm2p']:
                w_sb[name] = load(name + '_sb', [C + 1, C], di[name][:])
            mtap_sb = cpool.tile([C, 9 * C], bf16, name='mtap_sb')
            nc.sync.dma_start(mtap_sb[:], di['mtap'].ap().rearrange("t a b -> a t b"))
            xdw_sb = cpool.tile([C, 9 * Q], bf16, name='xdw_sb')
            nc.sync.dma_start(xdw_sb[:], di['xdw'].ap().rearrange("t c q -> c t q"))
            b_sb = {}
            for l in (1, 2):
                b_sb[l] = cpool.tile([128, HEADS * NCH * Q], bf16, name=f'b{l}_sb')
                nc.sync.dma_start(b_sb[l][:], di[f'b{l}'][:])

            # xT' (ones row and zero pad baked in on host)
            xT = cpool.tile([C + 1, NPAD], bf16, name='xT')
            nc.sync.dma_start(xT[:], di['xT'][:])
            xqT = cpool.tile([C + 1, Q], bf16, name='xqT')
            nc.sync.dma_start(xqT[:], di['xq'][:])

            # PE warmup burst: dense back-to-back matmuls flip HAM to 8/8
            ps_w = pss.tile([128, 512], f32, name='ps_w', tag='mm')
            for _ in range(48):
                nc.tensor.matmul(ps_w[:, 0:64], id_sb[:, 0:128],
                                 id_sb[:, 0:64], start=True, stop=True,
                                 skip_group_check=True)
            # preload exp table with a tiny op
            dummy = cpool.tile([1, 1], f32, name='dummy')
            nc.vector.memset(dummy[:], 0.0)
            dummy2 = cpool.tile([1, 1], f32, name='dummy2')
            nc.scalar.activation(dummy2[:], dummy[:], AF.Exp)

            x2T = cpool.tile([C + 1, NPAD], bf16, name='x2T')
            nc.vector.memset(x2T[:, N:], 0.0)
            nc.vector.memset(x2T[C:C + 1, :N], 1.0)

            y1T = cpool.tile([C + 1, Q], bf16, name='y1T')
            nc.vector.memset(y1T[C:C + 1, :], 1.0)

            def nat_layer(l, srcT, src_qT):
                """srcT: [65, NPAD] AP (full tokens, ch-major, ones row, zero pad);
                src_qT: [65, Q] AP.  Returns attnT' [65, Q] tile (ones row set)."""
                wq, wk, wv = w_sb[f'wq{l}'], w_sb[f'wk{l}'], w_sb[f'wv{l}']
                # q projection -> [128, Q], head h at partitions 32h..32h+16
                ps_q = pss.tile([128, 512], f32, name='ps_q', tag='mm')
                nc.tensor.matmul(ps_q[:, :Q], wq[:], src_qT, start=True, stop=True)
                qT = wpool.tile([128, Q], bf16, name='qT')
                nc.vector.tensor_copy(qT[:], ps_q[:, :Q])
                # k projection -> kT [128, NPAD]
                kT = wpool.tile([128, NPAD], bf16, name='kT')
                for jb, (s0, sz) in enumerate([(0, 512), (512, 512), (1024, 272)]):
                    ps_k = pss.tile([128, 512], f32, name='ps_k', tag='mm')
                    nc.tensor.matmul(ps_k[:, :sz], wk[:], srcT[:, s0:s0 + sz],
                                     start=True, stop=True)
                    nc.vector.tensor_copy(kT[:, s0:s0 + sz], ps_k[:, :sz])
                # v projection -> VV [128, nb*68 + 17h + d], ones col at 17h+16
                VV = wpool.tile([128, NCH * 68], bf16, name='VV')
                VVr = VV[:].rearrange("p (nb g d) -> p nb g d", g=HEADS, d=17)
                nc.vector.memset(VV[:], 0.0)
                nc.vector.memset(VVr[:, :, :, 16:17], 1.0)
                for nb in range(NCH):
                    nv = 128 if nb < NCH - 1 else N - 128 * (NCH - 1)
                    ps_v = pss.tile([128, 512], f32, name='ps_v', tag='mm')
                    nc.tensor.matmul(ps_v[:nv, :C],
                                     srcT[:, 128 * nb:128 * nb + nv],
                                     wv[:], start=True, stop=True)
                    nc.vector.tensor_copy(
                        VVr[:nv, nb, :, 0:16],
                        ps_v[:nv, :C].rearrange("p (g d) -> p g d", d=16))
                # S^T = K Q^T per head, + bias, exp -> PT
                PT = wpool.tile([128, HEADS * NCH * Q], bf16, name='PT')
                PTr = PT[:].rearrange("p (h nb q) -> p h nb q", h=HEADS, nb=NCH)
                Br = b_sb[l][:].rearrange("p (h nb q) -> p h nb q", h=HEADS, nb=NCH)
                ps_o = pss.tile([128, 512], f32, name='ps_o', tag='pair')
                for g0, gn in ((0, 3), (3, 3), (6, 3), (9, 2)):
                    stiles = []
                    for h in range(HEADS):
                        ps_s = psb.tile([128, 512], f32, name='ps_s', tag='s')
                        stiles.append(ps_s)
                    for i in range(gn):
                        nb = g0 + i
                        nv = 128 if nb < NCH - 1 else N - 128 * (NCH - 1)
                        for h in range(HEADS):
                            nc.tensor.matmul(
                                stiles[h][:nv, i * Q:i * Q + Q],
                                kT[32 * h:32 * h + 16, 128 * nb:128 * nb + nv],
                                qT[32 * h:32 * h + 16, :],
                                start=(i == 0), stop=False,
                                skip_group_check=True, tile_position=(32 * h, 0))
                    for h in range(HEADS):
                        nc.tensor.matmul(
                            stiles[h][:, 0:gn * Q],
                            id_sb[:],
                            Br[:, h, g0:g0 + gn, :],
                            start=False, stop=True,
                            skip_group_check=True)
                    for h in range(HEADS):
                        nc.scalar.activation(
                            PTr[:, h, g0:g0 + gn, :],
                            stiles[h][:, 0:gn * Q],
                            AF.Exp)
                    # PV for this group's chunks (overlaps next group's S-mms)
                    for i in range(gn):
                        nb = g0 + i
                        for h in range(HEADS):
                            nc.tensor.matmul(
                                ps_o[32 * h:32 * h + 17, :Q],
                                VVr[:, nb, h, :],
                                PTr[:, h, nb, :],
                                start=(nb == 0), stop=(nb == NCH - 1),
                                skip_group_check=True, tile_position=(0, 32 * h))
                # normalize: transpose -> divide -> transpose back
                o_sb = wpool.tile([128, Q], bf16, name='o_sb')
                nc.vector.tensor_copy(o_sb[:], ps_o[:, :Q])
                ps_t0 = pss.tile([128, 512], bf16, name='ps_t0', tag='pair')
                ps_t1 = pss.tile([128, 512], bf16, name='ps_t1', tag='pair')
                nc.tensor.transpose(ps_t0[:, :128], o_sb[:, 0:128], id_sb[:])
                nc.tensor.transpose(ps_t1[:34, :128], o_sb[:, 128:Q], id_sb[:])
                rec = wpool.tile([128, 8], f32, name='rec')
                t0v = ps_t0[:, :128].rearrange("p (h d) -> p h d", d=32)
                t1v = ps_t1[:34, :128].rearrange("p (h d) -> p h d", d=32)
                nc.vector.reciprocal(rec[:, 0:4], t0v[:, :, 16:17])
                nc.vector.reciprocal(rec[:34, 4:8], t1v[:, :, 16:17])
                aq0 = wpool.tile([128, C], bf16, name='aq0')
                aq1 = wpool.tile([34, C], bf16, name='aq1')
                for h in range(HEADS):
                    nc.vector.tensor_scalar_mul(
                        aq0[:, 16 * h:16 * h + 16],
                        ps_t0[:, 32 * h:32 * h + 16], rec[:, h:h + 1])
                    nc.vector.tensor_scalar_mul(
                        aq1[:, 16 * h:16 * h + 16],
                        ps_t1[:34, 32 * h:32 * h + 16], rec[:34, 4 + h:5 + h])
                ps_a0 = pss.tile([128, 512], bf16, name='ps_a0', tag='pair')
                ps_a1 = pss.tile([128, 512], bf16, name='ps_a1', tag='pair')
                nc.tensor.transpose(ps_a0[:C, :128], aq0[:], id_sb[:])
                nc.tensor.transpose(ps_a1[:C, :34], aq1[:], id_sb[:34, :34])
                attnT = wpool.tile([C + 1, Q], bf16, name=f'attnT{l}')
                nc.vector.memset(attnT[C:C + 1, :], 1.0)
                nc.scalar.copy(attnT[:C, 0:128], ps_a0[:C, :128])
                nc.scalar.copy(attnT[:C, 128:Q], ps_a1[:C, :34])
                return attnT

            # ---------------- layer 1 ----------------
            attnT1 = nat_layer(1, xT[:], xqT[:])
            ps_y = pss.tile([128, 512], f32, name='ps_y', tag='pair')
            nc.tensor.matmul(ps_y[:C, :Q], w_sb['proj1'][:], attnT1[:],
                             start=True, stop=True)
            nc.scalar.copy(y1T[:C, :], ps_y[:C, :Q])
            # all-gather y1 band
            nc.sync.dma_start(cc_in[:], y1T[:C, :])
            nc.gpsimd.collective_compute(
                "AllGather", mybir.AluOpType.bypass,
                replica_groups=[list(range(NCORES))],
                ins=[cc_in.ap().opt()], outs=[cc_out.ap().opt()])
            nc.sync.dma_start(x2T[:C, :N],
                              cc_out.ap().rearrange("r c q -> c r q"))
            # re-warm PE after the AllGather stall (deps on x2T place it there)
            ps_w2 = pss.tile([128, 512], f32, name='ps_w2', tag='mm')
            for _ in range(40):
                nc.tensor.matmul(ps_w2[:, 0:64], x2T[0:65, 0:128],
                                 x2T[0:65, 0:64], start=True, stop=True,
                                 skip_group_check=True)
            # ---------------- layer 2 ----------------
            attnT2 = nat_layer(2, x2T[:], y1T[:])
            # final: z^T = m2p @ attnT2' + sum_tap mtap @ xdw
            ps_z = pss.tile([128, 512], f32, name='ps_z', tag='pair')
            for t in range(9):
                nc.tensor.matmul(ps_z[:C, :Q],
                                 mtap_sb[:, C * t:C * t + C],
                                 xdw_sb[:, Q * t:Q * t + Q],
                                 start=(t == 0), stop=False, skip_group_check=True)
            nc.tensor.matmul(ps_z[:C, :Q], w_sb['m2p'][:], attnT2[:],
                             start=False, stop=True, skip_group_check=True)
            z_sb = wpool.tile([C, Q], bf16, name='z_sb')
            nc.scalar.copy(z_sb[:], ps_z[:C, :Q])
            ps_f0 = pss.tile([128, 512], bf16, name='ps_f0', tag='pair')
            ps_f1 = pss.tile([128, 512], bf16, name='ps_f1', tag='pair')
            nc.tensor.transpose(ps_f0[:, :C], z_sb[:, 0:128], id_sb[:C, :C])
            nc.tensor.transpose(ps_f1[:34, :C], z_sb[:, 128:Q], id_sb[:C, :C])
            zo0 = wpool.tile([128, C], f32, name='zo0')
            zo1 = wpool.tile([34, C], f32, name='zo1')
            nc.vector.tensor_copy(zo0[:], ps_f0[:, :C])
            nc.vector.tensor_copy(zo1[:], ps_f1[:34, :C])
            nc.sync.dma_start(out_d[0:128, :], zo0[:])
            nc.sync.dma_start(out_d[128:Q, :], zo1[:])

    nc.finalize()
    return nc


def kernel(**inputs) -> np.ndarray:
    from concourse.bass_utils import run_bass_kernel_spmd
    if 'nc' not in _CACHE:
        _CACHE['nc'] = _build_program()
    nc = _CACHE['nc']
    shared, percore = _prep(inputs)
    in_maps = []
    for core in range(NCORES):
        m = dict(shared)
        m.update(percore[core])
        in_maps.append(m)
    res = run_bass_kernel_spmd(nc, in_maps, core_ids=list(range(NCORES)))
    outs = [res.results[c]['out'] for c in range(NCORES)]
    full = np.concatenate(outs, axis=0).reshape(1, N, C)
    return full.astype(np.float32)


if __name__ == '__main__':
    import reference
    inputs = reference.setup_inputs()
    inputs = {k: np.asarray(v) for k, v in inputs.items()}
    got = kernel(**inputs)
    print("kernel output", got.shape, got.dtype)


# revision 18
# speedup vs baseline: 2.4761x; 1.1065x over previous
"""Trainium2 Bass kernel for nn_ConvNat (2x NeighborhoodAttention2D + dwconv + linear).

Strategy (8 NeuronCores, SPMD):
  - Dense masked attention instead of the 31x31 gather: every query attends to all
    1296 keys; a host-precomputed log-domain bias (rpb value in-window, -20 outside)
    is added to the logits via an identity-matmul accumulate into PSUM, then exp on
    the scalar engine.  No max-subtraction needed (logits are tiny); the denominator
    comes from an extra ones-column appended to V.
  - Channel-major ("transposed") layouts throughout so no device transposes are
    needed on the critical path; projections are affine-folded (bias rows appended
    to the stationary operands).
  - Cores shard the 1296 queries into 8 bands of 162.  K/V are computed replicated;
    the NAT1 output band is AllGathered between the layers.
  - The dwconv and its bias, proj2 bias, and lin bias are all folded into the final
    linear (host-precomputed 9 tap matrices + one combined matrix).
"""

import os
import numpy as np
import ml_dtypes

BF16 = ml_dtypes.bfloat16

HEADS = 4
K = 31
C = 64
DH = 16
H = W = 36
N = H * W            # 1296 tokens
NCORES = 8
Q = N // NCORES      # 162 queries per core
NCH = 11             # n-chunks of 128 (1296 -> 1408 padded)
NPAD = NCH * 128
MASKVAL = -20.0

_CACHE = {}


def _build_bias(rpb, t0):
    """Log-domain dense bias for queries [t0, t0+Q): shape (HEADS, NCH, 128, Q)."""
    n = np.arange(NPAD)
    r = np.minimum(n // 36, 35)          # clamp pad rows (masked anyway)
    c = n % 36 if True else None
    c = n % 36
    valid_n = (n < N)
    t = np.arange(t0, t0 + Q)
    i = t // 36
    j = t % 36
    ri = np.clip(i - K // 2, 0, H - K)   # (Q,)
    cj = np.clip(j - K // 2, 0, W - K)
    # in-window mask: ri <= r <= ri+30, cj <= c <= cj+30
    rm = (r[:, None] >= ri[None, :]) & (r[:, None] <= ri[None, :] + K - 1)
    cm = (c[:, None] >= cj[None, :]) & (c[:, None] <= cj[None, :] + K - 1)
    m = rm & cm & valid_n[:, None]       # (NPAD, Q)
    rrel = np.clip(r[:, None] - i[None, :] + (K - 1), 0, 2 * K - 2)
    crel = np.clip(c[:, None] - j[None, :] + (K - 1), 0, 2 * K - 2)
    bias = rpb[:, rrel, crel]            # (HEADS, NPAD, Q)
    bias = np.where(m[None], bias, MASKVAL).astype(np.float32)
    bias = bias.reshape(HEADS, NCH, 128, Q).transpose(2, 0, 1, 3)
    return np.ascontiguousarray(bias.reshape(128, HEADS * NCH * Q))


def _interleaved_wk(qkv_w, qkv_b, off, scale=1.0):
    """[65, 128] stationary: cols 32h+0..16 = head-h rows (off+16h+d) of qkv_w^T."""
    wt = np.zeros((C + 1, 128), np.float32)
    for h in range(HEADS):
        rows = qkv_w[off + 16 * h: off + 16 * h + 16]          # (16, 64)
        wt[:C, 32 * h: 32 * h + 16] = rows.T * scale
        wt[C, 32 * h: 32 * h + 16] = qkv_b[off + 16 * h: off + 16 * h + 16] * scale
    return wt


def _bf(a):
    return np.ascontiguousarray(np.asarray(a, np.float32).astype(BF16))


def _prep(inputs):
    x = np.asarray(inputs['x'], np.float32).reshape(N, C)
    p = {}
    xT = np.zeros((C + 1, NPAD), np.float32)
    xT[:C, :N] = x.T
    xT[C, :N] = 1.0
    p['xT'] = xT
    scale = DH ** -0.5
    for l, sfx in ((1, '1'), (2, '2')):
        qkv_w = np.asarray(inputs[f'qkv_w{sfx}'], np.float32)
        qkv_b = np.asarray(inputs[f'qkv_b{sfx}'], np.float32)
        p[f'wq{l}'] = _interleaved_wk(qkv_w, qkv_b, 0, scale)
        p[f'wk{l}'] = _interleaved_wk(qkv_w, qkv_b, C)
        wv = np.zeros((C + 1, C), np.float32)
        wv[:C] = qkv_w[2 * C:].T
        wv[C] = qkv_b[2 * C:]
        p[f'wv{l}'] = wv
    proj_w1 = np.asarray(inputs['proj_w1'], np.float32)
    proj_b1 = np.asarray(inputs['proj_b1'], np.float32)
    pr1 = np.zeros((C + 1, C), np.float32)
    pr1[:C] = proj_w1.T
    pr1[C] = proj_b1
    p['proj1'] = pr1
    lin_w = np.asarray(inputs['lin_w'], np.float32)
    lin_b = np.asarray(inputs['lin_b'], np.float32)
    proj_w2 = np.asarray(inputs['proj_w2'], np.float32)
    proj_b2 = np.asarray(inputs['proj_b2'], np.float32)
    dw_w = np.asarray(inputs['dw_w'], np.float32)               # (64, 1, 3, 3)
    dw_b = np.asarray(inputs['dw_b'], np.float32)
    m2 = np.zeros((C + 1, C), np.float32)
    m2[:C] = (lin_w @ proj_w2).T
    m2[C] = lin_w @ proj_b2 + lin_w @ dw_b + lin_b
    p['m2p'] = m2
    mtap = np.zeros((9, C, C), np.float32)
    for di in range(3):
        for dj in range(3):
            mtap[di * 3 + dj] = (lin_w * dw_w[None, :, 0, di, dj]).T  # diag(w)@lin_w^T
    # pack all small stationary tensors into one [128, 1472] tensor (one DMA)
    wpack = np.zeros((128, 1472), np.float32)
    wpack[:C + 1, 0:128] = p.pop('wq1')
    wpack[:C + 1, 128:256] = p.pop('wk1')
    wpack[:C + 1, 256:384] = p.pop('wq2')
    wpack[:C + 1, 384:512] = p.pop('wk2')
    wpack[:C + 1, 512:576] = p.pop('wv1')
    wpack[:C + 1, 576:640] = p.pop('wv2')
    wpack[:C + 1, 640:704] = p.pop('proj1')
    wpack[:C + 1, 704:768] = m2
    del p['m2p']
    wpack[:, 768:896] = np.eye(128, dtype=np.float32)
    wpack[:C, 896:1472] = mtap.transpose(1, 0, 2).reshape(C, 576)
    p['wpack'] = wpack
    # per-core tensors
    x_img = x.reshape(H, W, C).transpose(2, 0, 1)               # (64, 36, 36)
    xpad = np.zeros((C, H + 2, W + 2), np.float32)
    xpad[:, 1:-1, 1:-1] = x_img
    rpb1 = np.asarray(inputs['rpb1'], np.float32)
    rpb2 = np.asarray(inputs['rpb2'], np.float32)
    percore = []
    for core in range(NCORES):
        t0 = core * Q
        d = {}
        xq = np.zeros((C + 1, Q), np.float32)
        xq[:C] = x[t0:t0 + Q].T
        xq[C] = 1.0
        d['xq'] = xq
        xdw = np.zeros((9, C, Q), np.float32)
        for di in range(3):
            for dj in range(3):
                sh = xpad[:, di:di + H, dj:dj + W].reshape(C, N)
                xdw[di * 3 + dj] = sh[:, t0:t0 + Q]
        d['xdw'] = xdw
        d['b1'] = _build_bias(rpb1, t0)
        d['b2'] = _build_bias(rpb2, t0)
        percore.append(d)
    p = {k: _bf(v) for k, v in p.items()}
    percore = [{k: _bf(v) for k, v in d.items()} for d in percore]
    return p, percore


def _build_program():
    import concourse.bass as bass
    import concourse.bacc as bacc
    import concourse.tile as tile
    from concourse import mybir
    f32 = mybir.dt.float32
    bf16 = mybir.dt.bfloat16
    AF = mybir.ActivationFunctionType

    nc = bacc.Bacc("TRN2", target_bir_lowering=False, debug=False,
                   num_devices=NCORES)

    # ---- dram I/O ----
    di = {}
    for name, shape in [
        ('xT', [C + 1, NPAD]), ('xq', [C + 1, Q]), ('xdw', [9, C, Q]),
        ('wpack', [128, 1472]),
        ('b1', [128, HEADS * NCH * Q]), ('b2', [128, HEADS * NCH * Q]),
    ]:
        di[name] = nc.dram_tensor(name, shape, bf16, kind="ExternalInput")
    out_d = nc.dram_tensor('out', [Q, C], f32, kind="ExternalOutput")
    cc_in = nc.dram_tensor('cc_in', [C, Q], bf16)
    cc_out = nc.dram_tensor('cc_out', [NCORES, C, Q], bf16, addr_space="Shared")

    with tile.TileContext(nc) as tc:
        with (
            tc.tile_pool(name="const", bufs=1) as cpool,
            tc.tile_pool(name="work", bufs=2) as wpool,
            tc.tile_pool(name="ps_big", bufs=4, space="PSUM") as psb,
            tc.tile_pool(name="ps_small", bufs=2, space="PSUM") as pss,
        ):
            # ---- load constants (dependency-ordered, few big DMAs) ----
            wpack_sb = cpool.tile([128, 1472], bf16, name='wpack_sb')
            nc.sync.dma_start(wpack_sb[:], di['wpack'][:])
            xqT = cpool.tile([C + 1, Q], bf16, name='xqT')
            nc.sync.dma_start(xqT[:], di['xq'][:])
            xT = cpool.tile([C + 1, NPAD], bf16, name='xT')
            nc.sync.dma_start(xT[:], di['xT'][:])
            b_sb = {}
            for l in (1, 2):
                b_sb[l] = cpool.tile([128, HEADS * NCH * Q], bf16, name=f'b{l}_sb')
                nc.sync.dma_start(b_sb[l][:], di[f'b{l}'][:])
            xdw_sb = cpool.tile([C, 9 * Q], bf16, name='xdw_sb')
            nc.sync.dma_start(xdw_sb[:], di['xdw'].ap().rearrange("t c q -> c t q"))
            w_sb = {
                'wq1': wpack_sb[0:C + 1, 0:128],
                'wk1': wpack_sb[0:C + 1, 128:256],
                'wq2': wpack_sb[0:C + 1, 256:384],
                'wk2': wpack_sb[0:C + 1, 384:512],
                'wv1': wpack_sb[0:C + 1, 512:576],
                'wv2': wpack_sb[0:C + 1, 576:640],
                'proj1': wpack_sb[0:C + 1, 640:704],
                'm2p': wpack_sb[0:C + 1, 704:768],
            }
            id_sb = wpack_sb[0:128, 768:896]
            mtap_sb = wpack_sb[0:C, 896:1472]

            # PE warmup burst: dense back-to-back matmuls flip HAM to 8/8
            ps_w = pss.tile([128, 512], f32, name='ps_w', tag='mm')
            for _ in range(48):
                nc.tensor.matmul(ps_w[:, 0:64], id_sb[:, 0:128],
                                 id_sb[:, 0:64], start=True, stop=True,
                                 skip_group_check=True)
            # preload exp table with a tiny op
            dummy = cpool.tile([1, 1], f32, name='dummy')
            nc.vector.memset(dummy[:], 0.0)
            dummy2 = cpool.tile([1, 1], f32, name='dummy2')
            nc.scalar.activation(dummy2[:], dummy[:], AF.Exp)

            x2T = cpool.tile([C + 1, NPAD], bf16, name='x2T')
            nc.vector.memset(x2T[:, N:], 0.0)
            nc.vector.memset(x2T[C:C + 1, :N], 1.0)

            y1T = cpool.tile([C + 1, Q], bf16, name='y1T')
            nc.vector.memset(y1T[C:C + 1, :], 1.0)

            def nat_layer(l, srcT, src_qT):
                """srcT: [65, NPAD] AP (full tokens, ch-major, ones row, zero pad);
                src_qT: [65, Q] AP.  Returns attnT' [65, Q] tile (ones row set)."""
                wq, wk, wv = w_sb[f'wq{l}'], w_sb[f'wk{l}'], w_sb[f'wv{l}']
                # q projection -> [128, Q], head h at partitions 32h..32h+16
                ps_q = pss.tile([128, 512], f32, name='ps_q', tag='mm')
                nc.tensor.matmul(ps_q[:, :Q], wq, src_qT, start=True, stop=True)
                qT = wpool.tile([128, Q], bf16, name='qT')
                nc.vector.tensor_copy(qT[:], ps_q[:, :Q])
                # k projection -> kT [128, NPAD]
                kT = wpool.tile([128, NPAD], bf16, name='kT')
                for jb, (s0, sz) in enumerate([(0, 512), (512, 512), (1024, 272)]):
                    ps_k = pss.tile([128, 512], f32, name='ps_k', tag='mm')
                    nc.tensor.matmul(ps_k[:, :sz], wk, srcT[:, s0:s0 + sz],
                                     start=True, stop=True)
                    nc.vector.tensor_copy(kT[:, s0:s0 + sz], ps_k[:, :sz])
                # v projection -> VV [128, nb*68 + 17h + d], ones col at 17h+16
                VV = wpool.tile([128, NCH * 68], bf16, name='VV')
                VVr = VV[:].rearrange("p (nb g d) -> p nb g d", g=HEADS, d=17)
                nc.vector.memset(VV[:], 0.0)
                nc.vector.memset(VVr[:, :, :, 16:17], 1.0)
                for nb in range(NCH):
                    nv = 128 if nb < NCH - 1 else N - 128 * (NCH - 1)
                    ps_v = pss.tile([128, 512], f32, name='ps_v', tag='mm')
                    nc.tensor.matmul(ps_v[:nv, :C],
                                     srcT[:, 128 * nb:128 * nb + nv],
                                     wv, start=True, stop=True)
                    nc.vector.tensor_copy(
                        VVr[:nv, nb, :, 0:16],
                        ps_v[:nv, :C].rearrange("p (g d) -> p g d", d=16))
                # S^T = K Q^T per head, + bias, exp -> PT
                PT = wpool.tile([128, HEADS * NCH * Q], bf16, name='PT')
                PTr = PT[:].rearrange("p (h nb q) -> p h nb q", h=HEADS, nb=NCH)
                Br = b_sb[l][:].rearrange("p (h nb q) -> p h nb q", h=HEADS, nb=NCH)
                ps_o = pss.tile([128, 512], f32, name='ps_o', tag='pair')
                for g0, gn in ((0, 3), (3, 3), (6, 3), (9, 2)):
                    stiles = []
                    for h in range(HEADS):
                        ps_s = psb.tile([128, 512], f32, name='ps_s', tag='s')
                        stiles.append(ps_s)
                    for i in range(gn):
                        nb = g0 + i
                        nv = 128 if nb < NCH - 1 else N - 128 * (NCH - 1)
                        for h in range(HEADS):
                            nc.tensor.matmul(
                                stiles[h][:nv, i * Q:i * Q + Q],
                                kT[32 * h:32 * h + 16, 128 * nb:128 * nb + nv],
                                qT[32 * h:32 * h + 16, :],
                                start=(i == 0), stop=False,
                                skip_group_check=True, tile_position=(32 * h, 0))
                    for h in range(HEADS):
                        nc.tensor.matmul(
                            stiles[h][:, 0:gn * Q],
                            id_sb,
                            Br[:, h, g0:g0 + gn, :],
                            start=False, stop=True,
                            skip_group_check=True)
                    for h in range(HEADS):
                        nc.scalar.activation(
                            PTr[:, h, g0:g0 + gn, :],
                            stiles[h][:, 0:gn * Q],
                            AF.Exp)
                    # PV for this group's chunks (overlaps next group's S-mms)
                    for i in range(gn):
                        nb = g0 + i
                        for h in range(HEADS):
                            nc.tensor.matmul(
                                ps_o[32 * h:32 * h + 17, :Q],
                                VVr[:, nb, h, :],
                                PTr[:, h, nb, :],
                                start=(nb == 0), stop=(nb == NCH - 1),
                                skip_group_check=True, tile_position=(0, 32 * h))
                # normalize: transpose -> divide -> transpose back
                o_sb = wpool.tile([128, Q], bf16, name='o_sb')
                nc.vector.tensor_copy(o_sb[:], ps_o[:, :Q])
                ps_t0 = pss.tile([128, 512], bf16, name='ps_t0', tag='pair')
                ps_t1 = pss.tile([128, 512], bf16, name='ps_t1', tag='pair')
                nc.tensor.transpose(ps_t0[:, :128], o_sb[:, 0:128], id_sb)
                nc.tensor.transpose(ps_t1[:34, :128], o_sb[:, 128:Q], id_sb)
                rec = wpool.tile([128, 8], f32, name='rec')
                t0v = ps_t0[:, :128].rearrange("p (h d) -> p h d", d=32)
                t1v = ps_t1[:34, :128].rearrange("p (h d) -> p h d", d=32)
                nc.vector.reciprocal(rec[:, 0:4], t0v[:, :, 16:17])
                nc.vector.reciprocal(rec[:34, 4:8], t1v[:, :, 16:17])
                aq0 = wpool.tile([128, C], bf16, name='aq0')
                aq1 = wpool.tile([34, C], bf16, name='aq1')
                for h in range(HEADS):
                    nc.vector.tensor_scalar_mul(
                        aq0[:, 16 * h:16 * h + 16],
                        ps_t0[:, 32 * h:32 * h + 16], rec[:, h:h + 1])
                    nc.vector.tensor_scalar_mul(
                        aq1[:, 16 * h:16 * h + 16],
                        ps_t1[:34, 32 * h:32 * h + 16], rec[:34, 4 + h:5 + h])
                ps_a0 = pss.tile([128, 512], bf16, name='ps_a0', tag='pair')
                ps_a1 = pss.tile([128, 512], bf16, name='ps_a1', tag='pair')
                nc.tensor.transpose(ps_a0[:C, :128], aq0[:], id_sb)
                nc.tensor.transpose(ps_a1[:C, :34], aq1[:], id_sb[:34, :34])
                attnT = wpool.tile([C + 1, Q], bf16, name=f'attnT{l}')
                nc.vector.memset(attnT[C:C + 1, :], 1.0)
                nc.scalar.copy(attnT[:C, 0:128], ps_a0[:C, :128])
                nc.scalar.copy(attnT[:C, 128:Q], ps_a1[:C, :34])
                return attnT

            # ---------------- layer 1 ----------------
            attnT1 = nat_layer(1, xT[:], xqT[:])
            ps_y = pss.tile([128, 512], f32, name='ps_y', tag='pair')
            nc.tensor.matmul(ps_y[:C, :Q], w_sb['proj1'], attnT1[:],
                             start=True, stop=True)
            nc.scalar.copy(y1T[:C, :], ps_y[:C, :Q])
            # all-gather y1 band
            nc.sync.dma_start(cc_in[:], y1T[:C, :])
            nc.gpsimd.collective_compute(
                "AllGather", mybir.AluOpType.bypass,
                replica_groups=[list(range(NCORES))],
                ins=[cc_in.ap().opt()], outs=[cc_out.ap().opt()])
            nc.sync.dma_start(x2T[:C, :N],
                              cc_out.ap().rearrange("r c q -> c r q"))
            # re-warm PE after the AllGather stall (deps on x2T place it there)
            ps_w2 = pss.tile([128, 512], f32, name='ps_w2', tag='mm')
            for _ in range(40):
                nc.tensor.matmul(ps_w2[:, 0:64], x2T[0:65, 0:128],
                                 x2T[0:65, 0:64], start=True, stop=True,
                                 skip_group_check=True)
            # ---------------- layer 2 ----------------
            attnT2 = nat_layer(2, x2T[:], y1T[:])
            # final: z^T = m2p @ attnT2' + sum_tap mtap @ xdw
            ps_z = pss.tile([128, 512], f32, name='ps_z', tag='pair')
            for t in range(9):
                nc.tensor.matmul(ps_z[:C, :Q],
                                 mtap_sb[:, C * t:C * t + C],
                                 xdw_sb[:, Q * t:Q * t + Q],
                                 start=(t == 0), stop=False, skip_group_check=True)
            nc.tensor.matmul(ps_z[:C, :Q], w_sb['m2p'], attnT2[:],
                             start=False, stop=True, skip_group_check=True)
            z_sb = wpool.tile([C, Q], bf16, name='z_sb')
            nc.scalar.copy(z_sb[:], ps_z[:C, :Q])
            ps_f0 = pss.tile([128, 512], bf16, name='ps_f0', tag='pair')
            ps_f1 = pss.tile([128, 512], bf16, name='ps_f1', tag='pair')
            nc.tensor.transpose(ps_f0[:, :C], z_sb[:, 0:128], id_sb[:C, :C])
            nc.tensor.transpose(ps_f1[:34, :C], z_sb[:, 128:Q], id_sb[:C, :C])
            zo0 = wpool.tile([128, C], f32, name='zo0')
            zo1 = wpool.tile([34, C], f32, name='zo1')
            nc.vector.tensor_copy(zo0[:], ps_f0[:, :C])
            nc.vector.tensor_copy(zo1[:], ps_f1[:34, :C])
            nc.sync.dma_start(out_d[0:128, :], zo0[:])
            nc.sync.dma_start(out_d[128:Q, :], zo1[:])

    nc.finalize()
    return nc


def kernel(**inputs) -> np.ndarray:
    from concourse.bass_utils import run_bass_kernel_spmd
    if 'nc' not in _CACHE:
        _CACHE['nc'] = _build_program()
    nc = _CACHE['nc']
    shared, percore = _prep(inputs)
    in_maps = []
    for core in range(NCORES):
        m = dict(shared)
        m.update(percore[core])
        in_maps.append(m)
    res = run_bass_kernel_spmd(nc, in_maps, core_ids=list(range(NCORES)))
    outs = [res.results[c]['out'] for c in range(NCORES)]
    full = np.concatenate(outs, axis=0).reshape(1, N, C)
    return full.astype(np.float32)


if __name__ == '__main__':
    import reference
    inputs = reference.setup_inputs()
    inputs = {k: np.asarray(v) for k, v in inputs.items()}
    got = kernel(**inputs)
    print("kernel output", got.shape, got.dtype)
